# revision 1
# baseline (speedup 1.0000x reference)
"""Trainium2 Bass kernel for nn_Block_19301583028789.

Pipeline (per batch element): channel-mixing Linear -> erf-GELU -> S4D (FFT conv
in the reference; computed here as an exact chunked linear recurrence) -> FiLM
-> erf-GELU -> per-channel residual.

Sharding: data-parallel over batch B=16 across 8 cores (2 batches/core);
all parameters replicated.

S4D math: y = u * K + D*u with K[m] = 2 Re sum_n coef_n lam_n^m.  Split L into
C=128 chunks of T=128.  Per chunk: local causal conv = lower-tri Toeplitz
matmul; cross-chunk carry = rank-N apply of the complex mode state
S[n,c] = sum_{c'<=c} mu^{c-c'} Z[n,c'], Z = per-chunk Vandermonde summary
(matmul), mu = lam^T.  The state scan runs hierarchically on DVE
(radix-8 within-group, sequential across 16 groups, then combine).
"""

import numpy as np

import concourse.bass as bass
import concourse.tile as tile
import concourse.mybir as mybir
from concourse.bass_utils import run_bass_kernel_spmd

B, H, L = 16, 64, 16384
N, CD = 64, 32
T = 128
C = L // T          # 128 chunks
G1 = 8              # chunks per group (L1 radix)
NG = C // G1        # 16 groups
NCORES = 8
BLOC = B // NCORES  # 2
FP32 = mybir.dt.float32
BF16 = mybir.dt.bfloat16
AF = mybir.ActivationFunctionType

_CACHE = {}


def _split_tail_drain_waits(nc, max_waits=1):
    """Walrus TPB_CTRL lowering only accepts 1 sync-wait per Drain/NoOp; Tile's
    tail drain accumulates one wait per outstanding proc.  Hoist extras onto
    NoOps inserted right before the offending instruction."""
    for fn in nc.m.functions:
        for blk in fn.blocks:
            insts = blk.instructions
            i = 0
            while i < len(insts):
                inst = insts[i]
                si = inst.sync_info
                if si is not None and len(si.on_wait) > max_waits:
                    extra = list(si.on_wait[:-max_waits])
                    keep = list(si.on_wait[-max_waits:])
                    nops = [
                        mybir.InstNoOp(
                            name=f"{inst.name}-waitsplit{k}",
                            opcode="NoOp",
                            engine=inst.engine,
                            sync_info=mybir.SyncInfo(on_wait=[w], on_update=[]),
                        )
                        for k, w in enumerate(extra)
                    ]
                    si.on_wait = keep
                    for k, nop in enumerate(nops):
                        insts.insert(i + k, nop)
                    i += len(nops)
                i += 1


def _host_params(log_dt, log_A_real, A_imag, C_re, C_im, D, W_lin, b_lin):
    """Parameter-derived small constant matrices (fp64 host math)."""
    dt = np.exp(log_dt.astype(np.float64))[:, None]            # [H,1]
    A = -np.exp(log_A_real.astype(np.float64)) + 1j * A_imag.astype(np.float64)
    dtA = A * dt                                               # [H,N]
    coef = (C_re.astype(np.float64) + 1j * C_im.astype(np.float64)) \
        * (np.exp(dtA) - 1.0) / A                              # [H,N]

    # lam^k = exp(dtA*k), k = 0..T+1
    ks = np.arange(T + 2)
    lp = np.exp(dtA[:, :, None] * ks[None, None, :])           # [H,N,T+2]

    # K kernel first T taps; fold D into tap 0
    K = 2.0 * np.real(np.einsum("hn,hnm->hm", coef, lp[:, :, :T]))  # [H,T]
    K[:, 0] += D.astype(np.float64)

    # Toeplitz lhsT[j,t] = K[t-j] (t>=j)
    idx = np.arange(T)
    tm = idx[None, :] - idx[:, None]                           # [j,t]
    mask = tm >= 0
    Ktoep = np.where(mask, K[:, np.clip(tm, 0, T - 1)], 0.0)   # [H,j,t]

    # Z summary lhsT[t,n] = lam^(T-1-t)
    Alq = lp[:, :, ::-1][:, :, 2:T + 2]                        # lam^(T-1-t): [H,N,T] t-minor
    A_re = np.transpose(Alq.real, (0, 2, 1))                   # [H,T,N]
    A_im = np.transpose(Alq.imag, (0, 2, 1))

    # carry apply P[t,n] = 2*coef*lam^(t+1); lhsT [n,t]; im-part negated
    P = 2.0 * coef[:, :, None] * lp[:, :, 1:T + 1]             # [H,N,T]
    P_re = P.real                                              # [H,N,T] already [n,t]
    P_imn = -P.imag

    mu = lp[:, :, T]                                           # lam^T [H,N]
    nu = np.exp(dtA * T * G1)                                  # mu^G1
    comb = np.exp(dtA[:, :, None] * (T * (np.arange(1, G1 + 1))[None, None, :]))
    # comb[..,k] = mu^(k+1), k=0..G1-1

    f32 = lambda a: np.ascontiguousarray(a, dtype=np.float32)
    bf = lambda a: np.ascontiguousarray(a.astype(np.float32), dtype=np.float32)
    # scan coefficient tiles: [N(part), H] layout
    return {
        "Ktoep": f32(Ktoep),
        "A_re": f32(A_re), "A_im": f32(A_im),
        "P_re": bf(P_re), "P_imn": bf(P_imn),
        "mu_re": f32(mu.real.T), "mu_im": f32(mu.imag.T),          # [N,H]
        "nu_re": f32(nu.real.T), "nu_im": f32(nu.imag.T),
        "comb_re": f32(np.transpose(comb.real, (1, 2, 0))),        # [N,G1,H]
        "comb_im": f32(np.transpose(comb.imag, (1, 2, 0))),
        "WB": f32(np.concatenate([W_lin.T, b_lin[None, :]], 0)),   # [H+1,H]
    }


def _build(scan_dtype=BF16):
    nc = bass.Bass("TRN2", target_bir_lowering=False, debug=False)

    dram = {}
    def din(name, shape, dtype=FP32):
        dram[name] = nc.dram_tensor(name, list(shape), dtype, kind="ExternalInput")
        return dram[name]

    x_in = din("x_loc", [BLOC, H, L])
    WB = din("WB", [H + 1, H])
    Ktoep = din("Ktoep", [H, T, T])
    A_re = din("A_re", [H, T, N]); A_im = din("A_im", [H, T, N])
    P_re = din("P_re", [H, N, T], BF16); P_imn = din("P_imn", [H, N, T], BF16)
    mu_re = din("mu_re", [N, H], BF16); mu_im = din("mu_im", [N, H], BF16)
    nu_re = din("nu_re", [N, H], BF16); nu_im = din("nu_im", [N, H], BF16)
    comb_re = din("comb_re", [N, G1, H], BF16); comb_im = din("comb_im", [N, G1, H], BF16)
    eye = din("eye", [128, 128])
    film_WT = din("film_WT", [CD, 2 * H])
    film_bl = din("film_bl", [1, 2 * H])
    condT = din("condT", [CD, BLOC])
    ones1 = din("ones1", [1, BLOC])
    res_w_row = din("res_w_row", [1, H])
    gb_scratch = nc.dram_tensor("gb_scratch", [2 * H * BLOC], FP32)
    y_out = nc.dram_tensor("y_out", [BLOC, H, L], FP32, kind="ExternalOutput")

    xv = x_in.ap().rearrange("b h (c t) -> b h c t", t=T)
    yv = y_out.ap().rearrange("b h (c t) -> b h c t", t=T)

    with tile.TileContext(nc) as tc:
        with (
            tc.tile_pool(name="big", bufs=1) as big,
            tc.tile_pool(name="xhl", bufs=3) as xhl,
            tc.tile_pool(name="par", bufs=3) as par,
            tc.tile_pool(name="ev", bufs=3) as ev,
            tc.tile_pool(name="tmp", bufs=1) as tmp,
            tc.tile_pool(name="outp", bufs=3) as outp,
            tc.tile_pool(name="cst", bufs=1) as cst,
            tc.tile_pool(name="ps_w", bufs=2, space="PSUM") as ps_w,
            tc.tile_pool(name="ps_z", bufs=3, space="PSUM") as ps_z,
            tc.tile_pool(name="ps_y", bufs=2, space="PSUM") as ps_y,
            tc.tile_pool(name="ps_t", bufs=1, space="PSUM") as ps_t,
        ):
            # ---- resident tensors ----
            u = big.tile([128, H * BLOC * C], FP32, tag="u")      # [t,(h,b,c)]
            uv = u[:].rearrange("t (h b c) -> t h b c", h=H, b=BLOC)
            uc = u[:].rearrange("t (h b c) -> t b c h", h=H, b=BLOC)
            # scan state, re/im interleaved: [n, (h,b,g,c_rel,comp)]
            S = big.tile([N, H * BLOC * C * 2], scan_dtype, tag="S")
            Sv = S[:].rearrange("n (h b g r p) -> n h b g r p", h=H, b=BLOC, g=NG, r=G1)

            eye_sb = cst.tile([128, 128], FP32, tag="eye")
            nc.sync.dma_start(eye_sb[:], eye.ap())
            wb_sb = cst.tile([H + 1, H], FP32, tag="wb")
            nc.sync.dma_start(wb_sb[:], WB.ap())
            muT = cst.tile([N, 2 * H], BF16, tag="mu")
            nc.sync.dma_start(muT[:, 0:H], mu_re.ap())
            nc.sync.dma_start(muT[:, H:2 * H], mu_im.ap())
            nuT = cst.tile([N, 2 * H], BF16, tag="nu")
            nc.sync.dma_start(nuT[:, 0:H], nu_re.ap())
            nc.sync.dma_start(nuT[:, H:2 * H], nu_im.ap())
            combT = cst.tile([N, 2 * G1 * H], BF16, tag="comb")
            nc.sync.dma_start(
                combT[:, 0:G1 * H].rearrange("n (r h) -> n r h", r=G1), comb_re.ap())
            nc.sync.dma_start(
                combT[:, G1 * H:].rearrange("n (r h) -> n r h", r=G1), comb_im.ap())
            rwB = cst.tile([128, H], FP32, tag="rw")
            nc.sync.dma_start(rwB[:], res_w_row.ap().broadcast_to([128, H]))

            # ---- FiLM prologue: gb = film_W @ cond + film_b, broadcast ----
            fwt_sb = cst.tile([CD, 2 * H], FP32, tag="fwt")
            nc.sync.dma_start(fwt_sb[:], film_WT.ap())
            fbl_sb = cst.tile([1, 2 * H], FP32, tag="fbl")
            nc.sync.dma_start(fbl_sb[:], film_bl.ap())
            ct_sb = cst.tile([CD, BLOC], FP32, tag="ct")
            nc.sync.dma_start(ct_sb[:], condT.ap())
            on_sb = cst.tile([1, BLOC], FP32, tag="on")
            nc.sync.dma_start(on_sb[:], ones1.ap())
            gps = ps_z.tile([2 * H, BLOC], FP32, tag="z")
            nc.tensor.matmul(gps[:], fwt_sb[:], ct_sb[:], start=True, stop=False)
            nc.tensor.matmul(gps[:], fbl_sb[:], on_sb[:], start=False, stop=True)
            gb_sb = cst.tile([2 * H, BLOC], FP32, tag="gb")
            nc.scalar.copy(gb_sb[:], gps[:])
            nc.sync.dma_start(gb_scratch.ap().rearrange("(r b) -> r b", b=BLOC), gb_sb[:])
            gbB = cst.tile([128, 2 * H * BLOC], FP32, tag="gbB")
            nc.sync.dma_start(
                gbB[:], gb_scratch.ap().rearrange("f -> f")[None, :].broadcast_to(
                    [128, 2 * H * BLOC]))

            # ---- phase A: u = gelu(W x + b), transposed to [t,(h,b,c)] ----
            XC = 4  # c-tiles per x DMA
            for b in range(BLOC):
                for c0 in range(0, C, XC):
                    xt = xhl.tile([H + 1, XC * T], FP32, tag="xt")
                    nc.sync.dma_start(
                        xt[0:H, :].rearrange("h (c t) -> h c t", c=XC),
                        xv[b, :, c0:c0 + XC, :])
                    nc.vector.memset(xt[H:H + 1, :], 1.0)
                    for cc in range(0, XC, 2):
                        c = c0 + cc
                        wp = ps_w.tile([T, 2 * H], FP32)
                        nc.tensor.matmul(
                            wp[:, 0:H], xt[:, cc * T:(cc + 1) * T], wb_sb[:],
                            start=True, stop=True, skip_group_check=True)
                        nc.tensor.matmul(
                            wp[:, H:2 * H], xt[:, (cc + 1) * T:(cc + 2) * T], wb_sb[:],
                            start=True, stop=True, skip_group_check=True)
                        wpv = wp[:].rearrange("t (c h) -> t c h", c=2)
                        ucv = uc[:, b, c:c + 2, :]
                        nc.scalar.activation(ucv, wpv, AF.Gelu)

            # ---- phases B/C/D/E pipelined over two h-halves: the DVE scan of
            # one half overlaps the PE/ACT work of the other ----
            mre = muT[:, 0:H]; mim = muT[:, H:2 * H]
            nre = nuT[:, 0:H]; nim = nuT[:, H:2 * H]
            cv = combT[:].rearrange("n (p r h) -> n p r h", p=2, r=G1)

            def cfma(dst_re, dst_im, w_re, w_im, s_re, s_im, fshape):
                # dst += w * s (complex), w broadcast along trailing dims
                t1 = tmp.tile([N, 2048], BF16, tag="t1")
                t2 = tmp.tile([N, 2048], BF16, tag="t2")
                nf = int(np.prod(fshape))
                a = t1[:, :nf].rearrange("n (x y z) -> n x y z", x=fshape[0], y=fshape[1])
                bq = t2[:, :nf].rearrange("n (x y z) -> n x y z", x=fshape[0], y=fshape[1])
                wr = w_re.broadcast_to([N, *fshape])
                wi = w_im.broadcast_to([N, *fshape])
                nc.vector.tensor_mul(a, wr, s_re)
                nc.vector.tensor_mul(bq, wi, s_im)
                nc.vector.tensor_sub(a, a, bq)
                nc.vector.tensor_add(dst_re, dst_re, a)
                nc.vector.tensor_mul(a, wr, s_im)
                nc.vector.tensor_mul(bq, wi, s_re)
                nc.vector.tensor_add(a, a, bq)
                nc.vector.tensor_add(dst_im, dst_im, a)

            NSPLIT = 4
            HH = H // NSPLIT
            Sflat = S[:].rearrange("n (h b c p) -> n h b c p", h=H, b=BLOC, c=C)
            for half in range(NSPLIT):
                h0, h1 = half * HH, (half + 1) * HH

                # -- B: Z summaries for this half --
                for h in range(h0, h1):
                    are = par.tile([T, N], FP32, tag="are")
                    nc.sync.dma_start(are[:], A_re.ap()[h])
                    aim = par.tile([T, N], FP32, tag="aim")
                    nc.sync.dma_start(aim[:], A_im.ap()[h])
                    rhs = uv[:, h, :, :]  # [t,(b,c)] 3D ap
                    zr = ps_z.tile([N, BLOC * C], FP32, tag="z")
                    nc.tensor.matmul(zr[:], are[:], rhs, start=True, stop=True)
                    zi = ps_z.tile([N, BLOC * C], FP32, tag="z")
                    nc.tensor.matmul(zi[:], aim[:], rhs, start=True, stop=True)
                    zrv = zr[:].rearrange("n (b c) -> n b c", b=BLOC)
                    ziv = zi[:].rearrange("n (b c) -> n b c", b=BLOC)
                    for b in range(BLOC):
                        nc.scalar.copy(
                            Sv[:, h, b, :, :, 0].rearrange("n g r -> n (g r)"), zrv[:, b, :])
                        nc.scalar.copy(
                            Sv[:, h, b, :, :, 1].rearrange("n g r -> n (g r)"), ziv[:, b, :])

                # -- C: hierarchical scan for this half (DVE only) --
                hs = slice(h0, h1)
                for r in range(1, G1):
                    cfma(Sv[:, hs, :, :, r, 0], Sv[:, hs, :, :, r, 1],
                         mre[:, hs, None, None], mim[:, hs, None, None],
                         Sv[:, hs, :, :, r - 1, 0], Sv[:, hs, :, :, r - 1, 1],
                         (HH, BLOC, NG))
                for g in range(1, NG):
                    cfma(Sv[:, hs, :, g, G1 - 1, 0], Sv[:, hs, :, g, G1 - 1, 1],
                         nre[:, hs, None], nim[:, hs, None],
                         Sv[:, hs, :, g - 1, G1 - 1, 0], Sv[:, hs, :, g - 1, G1 - 1, 1],
                         (HH, BLOC, 1))
                for r in range(G1 - 1):
                    cfma(Sv[:, hs, :, 1:NG, r, 0], Sv[:, hs, :, 1:NG, r, 1],
                         cv[:, 0, r, hs, None, None], cv[:, 1, r, hs, None, None],
                         Sv[:, hs, :, 0:NG - 1, G1 - 1, 0], Sv[:, hs, :, 0:NG - 1, G1 - 1, 1],
                         (HH, BLOC, NG - 1))

                # -- D/E: toeplitz + carry apply + output assembly for this half --
                for h in range(h0, h1):
                    kt = par.tile([T, T], FP32, tag="kt")
                    nc.sync.dma_start(kt[:], Ktoep.ap()[h])
                    pre = par.tile([N, T], BF16, tag="pre")
                    nc.sync.dma_start(pre[:], P_re.ap()[h])
                    pim = par.tile([N, T], BF16, tag="pim")
                    nc.sync.dma_start(pim[:], P_imn.ap()[h])
                    yp = ps_y.tile([T, BLOC * C], FP32)
                    ypv = yp[:].rearrange("t (b c) -> t b c", b=BLOC)
                    nc.tensor.matmul(yp[:], kt[:], uv[:, h, :, :],
                                     start=True, stop=False)
                    nc.tensor.matmul(ypv[:, :, 1:C], pre[:],
                                     Sflat[:, h, :, 0:C - 1, 0],
                                     start=False, stop=False)
                    nc.tensor.matmul(ypv[:, :, 1:C], pim[:],
                                     Sflat[:, h, :, 0:C - 1, 1],
                                     start=False, stop=True)
                    z1 = ev.tile([T, BLOC * C], FP32, tag="z1")
                    nc.scalar.copy(z1[:], yp[:])
                    z1v = z1[:].rearrange("t (b c) -> t b c", b=BLOC)
                    for b in range(BLOC):
                        tp = ps_t.tile([128, 128], FP32)
                        nc.tensor.transpose(tp[:], z1v[:, b, :], eye_sb[:])
                        yt = outp.tile([128, T], FP32, tag="yt")
                        nc.scalar.activation(
                            yt[:], tp[:], AF.Gelu,
                            bias=gbB[:, (H + h) * BLOC + b:(H + h) * BLOC + b + 1],
                            scale=gbB[:, h * BLOC + b:h * BLOC + b + 1])
                        xc = outp.tile([128, T], FP32, tag="xc")
                        nc.sync.dma_start(xc[:], xv[b, h, :, :])
                        nc.vector.tensor_scalar_mul(xc[:], xc[:], rwB[:, h:h + 1])
                        nc.vector.tensor_add(yt[:], yt[:], xc[:])
                        nc.sync.dma_start(yv[b, h, :, :], yt[:])

    _split_tail_drain_waits(nc)
    return nc


def kernel(**inputs):
    key = "k"
    if key not in _CACHE:
        _CACHE[key] = _build()
    nc = _CACHE[key]

    hp = _host_params(
        inputs["log_dt"], inputs["log_A_real"], inputs["A_imag"],
        inputs["C_re"], inputs["C_im"], inputs["D"],
        inputs["W_lin"], inputs["b_lin"])

    x = np.ascontiguousarray(inputs["x"], dtype=np.float32)
    cond = np.ascontiguousarray(inputs["conditional_information"], dtype=np.float32)
    film_W = np.ascontiguousarray(inputs["film_W"], dtype=np.float32)
    film_b = np.ascontiguousarray(inputs["film_b"], dtype=np.float32)
    res_w = np.ascontiguousarray(inputs["res_w"], dtype=np.float32)

    bf = lambda a: np.ascontiguousarray(a, dtype=np.float32).astype(
        np.dtype("bfloat16") if False else np.float32)
    import ml_dtypes
    tobf = lambda a: np.ascontiguousarray(a.astype(ml_dtypes.bfloat16))

    common = {
        "WB": hp["WB"], "Ktoep": hp["Ktoep"],
        "A_re": hp["A_re"], "A_im": hp["A_im"],
        "P_re": tobf(hp["P_re"]), "P_imn": tobf(hp["P_imn"]),
        "mu_re": tobf(hp["mu_re"]), "mu_im": tobf(hp["mu_im"]),
        "nu_re": tobf(hp["nu_re"]), "nu_im": tobf(hp["nu_im"]),
        "comb_re": tobf(hp["comb_re"]), "comb_im": tobf(hp["comb_im"]),
        "eye": np.eye(128, dtype=np.float32),
        "film_WT": np.ascontiguousarray(film_W.T),
        "film_bl": film_b[None, :],
        "ones1": np.ones((1, BLOC), np.float32),
        "res_w_row": res_w[None, :],
    }
    in_maps = []
    for c_ in range(NCORES):
        m = dict(common)
        m["x_loc"] = np.ascontiguousarray(x[c_ * BLOC:(c_ + 1) * BLOC])
        m["condT"] = np.ascontiguousarray(cond[c_ * BLOC:(c_ + 1) * BLOC].T)
        in_maps.append(m)

    res = run_bass_kernel_spmd(nc, in_maps, core_ids=list(range(NCORES)))
    out = np.concatenate([res.results[c_]["y_out"] for c_ in range(NCORES)], axis=0)
    return out.astype(np.float32)



# revision 18
# speedup vs baseline: 2.3257x; 2.3257x over previous
"""Trainium2 Bass kernel for nn_Block_19301583028789 (v2).

Pipeline per batch: channel Linear -> erf-GELU -> S4D (chunked linear
recurrence, exact) -> FiLM -> erf-GELU -> per-channel residual.

v2 redesign vs v1 (587us):
- all matmuls bf16 (1 cyc/row), x pre-cast to bf16 host-side
- W=2 pair-level state scan: 64 scanned states instead of 128 (pair
  summaries built by accumulating matmuls at no extra PE cost)
- [c,t]-form conv output (out[c,t] = u^T KT + S^T P), chunk parity in the
  free dim -> no PE transposes, no extra PSUM->SBUF copy passes
- re/im carry contraction stacked on 128 partitions (1 matmul, not 2);
  im half moved across lanes by one SBUF->SBUF DMA after the scan
- few, large DMAs (HWDGE fixed cost is ~625ns per DMA instruction)
- residual pre-scaled by res_w host-side; bf16 output, host upcasts

Sharding: data-parallel over batch B=16 across 8 cores (2 per core).
"""

import numpy as np

import concourse.bass as bass
import concourse.tile as tile
import concourse.mybir as mybir
from concourse.bass_utils import run_bass_kernel_spmd

B, H, L = 16, 64, 16384
N, CD = 64, 32
T = 128
C = L // T            # 128 chunks
C2 = C // 2           # 64 chunk pairs (scan granularity)
G1 = 8                # pairs per scan group
NG = C2 // G1         # 8 groups
NCORES = 8
BLOC = B // NCORES    # 2
SCOL = C2 + 1         # pad column + 64 pair states
FP32 = mybir.dt.float32
BF16 = mybir.dt.bfloat16
AF = mybir.ActivationFunctionType

_CACHE = {}


def _split_tail_drain_waits(nc, max_waits=1):
    """Walrus TPB_CTRL lowering only accepts 1 sync-wait per Drain/NoOp."""
    for fn in nc.m.functions:
        for blk in fn.blocks:
            insts = blk.instructions
            i = 0
            while i < len(insts):
                inst = insts[i]
                si = inst.sync_info
                if si is not None and len(si.on_wait) > max_waits:
                    extra = list(si.on_wait[:-max_waits])
                    keep = list(si.on_wait[-max_waits:])
                    nops = [
                        mybir.InstNoOp(
                            name=f"{inst.name}-waitsplit{k}",
                            opcode="NoOp",
                            engine=inst.engine,
                            sync_info=mybir.SyncInfo(on_wait=[w], on_update=[]),
                        )
                        for k, w in enumerate(extra)
                    ]
                    si.on_wait = keep
                    for k, nop in enumerate(nops):
                        insts.insert(i + k, nop)
                    i += len(nops)
                i += 1


def _host_params(log_dt, log_A_real, A_imag, C_re, C_im, D, W_lin, b_lin):
    """Parameter-derived constant matrices (fp64 host math), bf16-packed."""
    import ml_dtypes
    bf = lambda a: np.ascontiguousarray(
        np.asarray(a, dtype=np.float64).astype(np.float32).astype(ml_dtypes.bfloat16))

    dt = np.exp(log_dt.astype(np.float64))[:, None]            # [H,1]
    A = -np.exp(log_A_real.astype(np.float64)) + 1j * A_imag.astype(np.float64)
    dtA = A * dt                                               # [H,N]
    coef = (C_re.astype(np.float64) + 1j * C_im.astype(np.float64)) \
        * (np.exp(dtA) - 1.0) / A                              # [H,N]

    ks = np.arange(2 * T + 2)
    lp = np.exp(dtA[:, :, None] * ks[None, None, :])           # [H,N,2T+2]

    # local-conv kernel, D folded into tap 0
    K = 2.0 * np.real(np.einsum("hn,hnm->hm", coef, lp[:, :, :T]))  # [H,T]
    K[:, 0] += D.astype(np.float64)
    # KT[t', h, t] = K_h[t - t'] for t >= t'   (rhs of out[c,t] local conv)
    idx = np.arange(T)
    dmat = idx[None, :] - idx[:, None]                         # [t', t]
    KT = np.where(dmat[None] >= 0, K[:, np.clip(dmat, 0, T - 1)], 0.0)  # [H,t',t]
    KT = np.transpose(KT, (1, 0, 2))                           # [t',H,t]
    # KT2: kernel taps T..2T-1 = carry from the immediately-preceding chunk
    # (K2[d] = 2 Re sum_n coef lam^d, d = T + t - t', dense)
    K2 = 2.0 * np.real(np.einsum("hn,hnm->hm", coef, lp[:, :, :2 * T]))  # [H,2T]
    KT2 = K2[:, T + dmat]                                      # [H,t',t]
    KT2 = np.transpose(KT2, (1, 0, 2))

    # pair-summary lhsT tables (V[p] = sum over 2 chunks of lam^(2T-1-tau) u)
    VA_lo = np.transpose(lp[:, :, ::-1][:, :, 2:T + 2], (2, 0, 1))      # lam^(2T-1-t) [t,H,N]
    VA_hi = np.transpose(lp[:, :, ::-1][:, :, T + 2:2 * T + 2], (2, 0, 1))  # lam^(T-1-t)

    # carry rhs, stacked (re | -im): y[c,t] += Re(sum_n S[n] P[n,t])
    P = 2.0 * coef[:, :, None] * lp[:, :, 1:T + 1]             # [H,N,T]
    mu = lp[:, :, T]                                           # lam^T [H,N]
    Pmu = P * mu[:, :, None]
    PST = np.concatenate([P.real, -P.imag], axis=1)            # [H,2N,T]
    PMT = np.concatenate([Pmu.real, -Pmu.imag], axis=1)
    PST = np.transpose(PST, (1, 0, 2))                         # [2N,H,T]
    PMT = np.transpose(PMT, (1, 0, 2))

    # scan coefficients over pairs: ratio nu = mu^2 = lam^(2T)
    nu = lp[:, :, 2 * T]                                       # [H,N]
    nuP = nu[None, :, :] ** (1 + np.arange(G1)[:, None, None]) # [G1,H,N]
    MU2_re = nu.real.T; MU2_im = nu.imag.T                     # [N,H]
    NU2_re = nuP[G1 - 1].real.T; NU2_im = nuP[G1 - 1].imag.T   # nu^G1
    CB_re = np.transpose(nuP[:G1 - 1].real, (2, 0, 1))         # [N,G1-1,H]
    CB_im = np.transpose(nuP[:G1 - 1].imag, (2, 0, 1))

    WB = np.concatenate([W_lin.T.astype(np.float64),
                         b_lin.astype(np.float64)[None, :]], 0)  # [H+1,H]

    return {
        "WB": bf(WB), "KT": bf(KT), "KT2": bf(KT2),
        "VA_lo_re": bf(VA_lo.real), "VA_lo_im": bf(VA_lo.imag),
        "VA_hi_re": bf(VA_hi.real), "VA_hi_im": bf(VA_hi.imag),
        "PST": bf(PST), "PMT": bf(PMT),
        "MU2_re": bf(MU2_re), "MU2_im": bf(MU2_im),
        "NU2_re": bf(NU2_re), "NU2_im": bf(NU2_im),
        "CB_re": bf(CB_re), "CB_im": bf(CB_im),
    }


def _build():
    nc = bass.Bass("TRN2", target_bir_lowering=False, debug=False)

    def din(name, shape, dtype=BF16):
        return nc.dram_tensor(name, list(shape), dtype, kind="ExternalInput")

    xbf = din("xbf", [BLOC, H + 1, L])           # x bf16 + ones row (phase A)
    xrt = din("xrt", [BLOC, C2, H, 2 * T])       # res_w*x, [b, c2, h, 256] bf16
    WB = din("WB", [H + 1, H])
    KT = din("KT", [T, H, T])
    KT2 = din("KT2", [T, H, T])
    VA_lo_re = din("VA_lo_re", [T, H, N]); VA_lo_im = din("VA_lo_im", [T, H, N])
    VA_hi_re = din("VA_hi_re", [T, H, N]); VA_hi_im = din("VA_hi_im", [T, H, N])
    PST = din("PST", [2 * N, H, T]); PMT = din("PMT", [2 * N, H, T])
    MU2_re = din("MU2_re", [N, H]); MU2_im = din("MU2_im", [N, H])
    NU2_re = din("NU2_re", [N, H]); NU2_im = din("NU2_im", [N, H])
    CB_re = din("CB_re", [N, G1 - 1, H]); CB_im = din("CB_im", [N, G1 - 1, H])
    film_WT = din("film_WT", [CD, 2 * H], FP32)
    film_bl = din("film_bl", [1, 2 * H], FP32)
    condT = din("condT", [CD, BLOC], FP32)
    ones1 = din("ones1", [1, BLOC], FP32)
    gb_scratch = nc.dram_tensor("gb_scratch", [2 * H * BLOC], FP32)
    y_out = nc.dram_tensor("y_out", [BLOC, H, L], BF16, kind="ExternalOutput")

    HGB = 4                      # h batch for phase B psum
    HG = 8                       # h batch for phase D store
    with tile.TileContext(nc) as tc:
        with (
            tc.tile_pool(name="big", bufs=1) as big,
            tc.tile_pool(name="cpar", bufs=1) as cpar,
            tc.tile_pool(name="xa", bufs=3) as xa,
            tc.tile_pool(name="tmp", bufs=1) as tmp,
            tc.tile_pool(name="yt", bufs=2) as ytp,
            tc.tile_pool(name="rx", bufs=2) as rxp,
            tc.tile_pool(name="ps_a", bufs=2, space="PSUM") as ps_a,
            tc.tile_pool(name="ps_b", bufs=2, space="PSUM") as ps_b,
            tc.tile_pool(name="ps_d", bufs=4, space="PSUM") as ps_d,
        ):
            # ---------------- resident tensors ----------------
            u = big.tile([T, H * BLOC * C], BF16, tag="u")      # [t,(h,b,c)]
            uv = u[:].rearrange("t (h b c) -> t h b c", h=H, b=BLOC)
            uq = u[:].rearrange("t (h b p q) -> t h b p q", h=H, b=BLOC, q=2)
            # stacked carry lhsT: rows 0:64 S_re, 64:128 S_im; cols (b, pad+p, h)
            Sst = big.tile([2 * N, BLOC * SCOL * H], BF16, tag="Sst")
            Sstv = Sst[:].rearrange("n (b p h) -> n b p h", b=BLOC, p=SCOL)
            # im-scratch on lanes 0:64 (scan runs here; DMA'd to Sst[64:128])
            Sim = big.tile([N, BLOC * C2 * H], BF16, tag="Sim")
            Simv = Sim[:].rearrange("n (b p h) -> n b p h", b=BLOC, p=C2)

            for b in range(BLOC):   # zero pad columns (both halves)
                nc.vector.memset(Sstv[:, b, 0, :], 0.0)

            # ---------------- parameters (persistent) ----------------
            wb_sb = cpar.tile([H + 1, H], BF16, tag="wb")
            nc.sync.dma_start(wb_sb[:], WB.ap())
            kt_sb = cpar.tile([T, H * T], BF16, tag="kt")
            nc.sync.dma_start(kt_sb[:].rearrange("t (h f) -> t h f", h=H), KT.ap())
            kt2_sb = cpar.tile([T, H * T], BF16, tag="kt2")
            nc.sync.dma_start(kt2_sb[:].rearrange("t (h f) -> t h f", h=H),
                              KT2.ap())
            va_sb = {}
            for nm, tens in (("lr", VA_lo_re), ("hr", VA_hi_re),
                             ("li", VA_lo_im), ("hi", VA_hi_im)):
                t_ = cpar.tile([T, H * N], BF16, tag="va" + nm)
                nc.sync.dma_start(t_[:].rearrange("t (h f) -> t h f", h=H),
                                  tens.ap())
                va_sb[nm] = t_
            pst_sb = cpar.tile([2 * N, H * T], BF16, tag="pst")
            nc.sync.dma_start(pst_sb[:].rearrange("n (h f) -> n h f", h=H),
                              PST.ap())
            pmt_sb = cpar.tile([2 * N, H * T], BF16, tag="pmt")
            nc.sync.dma_start(pmt_sb[:].rearrange("n (h f) -> n h f", h=H),
                              PMT.ap())
            mu2 = cpar.tile([N, 2 * H], BF16, tag="mu2")
            nc.sync.dma_start(mu2[:, 0:H], MU2_re.ap())
            nc.sync.dma_start(mu2[:, H:], MU2_im.ap())
            nu2 = cpar.tile([N, 2 * H], BF16, tag="nu2")
            nc.sync.dma_start(nu2[:, 0:H], NU2_re.ap())
            nc.sync.dma_start(nu2[:, H:], NU2_im.ap())
            cb = cpar.tile([N, 2 * (G1 - 1) * H], BF16, tag="cb")
            cbv = cb[:].rearrange("n (q r h) -> n q r h", q=2, r=G1 - 1)
            nc.sync.dma_start(cbv[:, 0], CB_re.ap())
            nc.sync.dma_start(cbv[:, 1], CB_im.ap())

            # FiLM prologue -> per-(h,b) scalar columns
            fwt = cpar.tile([CD, 2 * H], FP32, tag="fwt")
            nc.sync.dma_start(fwt[:], film_WT.ap())
            fbl = cpar.tile([1, 2 * H], FP32, tag="fbl")
            nc.sync.dma_start(fbl[:], film_bl.ap())
            ct = cpar.tile([CD, BLOC], FP32, tag="ct")
            nc.sync.dma_start(ct[:], condT.ap())
            on1 = cpar.tile([1, BLOC], FP32, tag="on1")
            nc.sync.dma_start(on1[:], ones1.ap())
            gps_t = ps_d.tile([128, 2 * T], FP32, tag="pd")
            gps = gps_t[0:2 * H, 0:BLOC]
            nc.tensor.matmul(gps, fwt[:], ct[:], start=True, stop=False)
            nc.tensor.matmul(gps, fbl[:], on1[:], start=False, stop=True)
            gb_sb = cpar.tile([2 * H, BLOC], FP32, tag="gb")
            nc.scalar.copy(gb_sb[:], gps)
            nc.sync.dma_start(
                gb_scratch.ap().rearrange("(r b) -> r b", b=BLOC), gb_sb[:])
            gbB = cpar.tile([N, 2 * H * BLOC], FP32, tag="gbB")
            nc.sync.dma_start(
                gbB[:], gb_scratch.ap().rearrange("f -> f")[None, :]
                .broadcast_to([N, 2 * H * BLOC]))

            # ---------------- phase A: u = gelu(W x + b) ----------------
            XC = 8      # chunks per x DMA; 4 chunks per psum tile
            for b in range(BLOC):
                for cg in range(C // XC):
                    xt = xa.tile([H + 1, XC * T], BF16, tag="xt")
                    nc.sync.dma_start(
                        xt[:], xbf.ap()[b, :, cg * XC * T:(cg + 1) * XC * T])
                    for half in range(2):
                        pa = ps_a.tile([T, 4 * H], FP32)
                        for cc in range(4):
                            cx = half * 4 + cc
                            nc.tensor.matmul(
                                pa[:, cc * H:(cc + 1) * H],
                                xt[:, cx * T:(cx + 1) * T], wb_sb[:],
                                start=True, stop=True, skip_group_check=True)
                        c0 = cg * XC + half * 4
                        dst = uv[:, :, b, c0:c0 + 4].rearrange("t h c -> t c h")
                        nc.scalar.activation(dst, pa[:], AF.Gelu)

            # ---------------- per-b: phase B + scan + phase D ----------------
            mre = mu2[:, 0:H]; mim = mu2[:, H:]
            nre = nu2[:, 0:H]; nim = nu2[:, H:]

            def cfma(dre, dim_, wre, wim, sre, sim_, fshape):
                """d += w * s (complex); w broadcast tiles, all bf16."""
                nf = int(np.prod(fshape))
                t1 = tmp.tile([N, NG * H], BF16, tag="t1")
                t2 = tmp.tile([N, NG * H], BF16, tag="t2")
                a = t1[:, :nf].rearrange("n (x y) -> n x y", x=fshape[0])
                bq = t2[:, :nf].rearrange("n (x y) -> n x y", x=fshape[0])
                nc.vector.tensor_mul(a, wre, sre)
                nc.vector.tensor_mul(bq, wim, sim_)
                nc.vector.tensor_sub(a, a, bq)
                nc.vector.tensor_add(dre, dre, a)
                nc.vector.tensor_mul(a, wre, sim_)
                nc.vector.tensor_mul(bq, wim, sre)
                nc.vector.tensor_add(a, a, bq)
                nc.vector.tensor_add(dim_, dim_, a)

            for b in range(BLOC):
                # scan views [n, g, r, h]
                sreV = Sstv[0:N, b, 1:SCOL, :].rearrange(
                    "n (g r) h -> n g r h", g=NG)
                simV = Simv[:, b, :, :].rearrange("n (g r) h -> n g r h", g=NG)

                # ---- phase B: V pair-summaries ----
                for hg in range(H // HGB):
                    h0 = hg * HGB
                    pv = ps_b.tile([N, 2 * HGB * C2], FP32)   # (q, h, p)
                    for hh in range(HGB):
                        h = h0 + hh
                        rhs_e = uq[:, h, b, :, 0]
                        rhs_o = uq[:, h, b, :, 1]
                        for q, (lo, hi) in enumerate(
                                (("lr", "hr"), ("li", "hi"))):
                            sl = slice((q * HGB + hh) * C2,
                                       (q * HGB + hh + 1) * C2)
                            vlo = va_sb[lo][:, h * N:(h + 1) * N]
                            vhi = va_sb[hi][:, h * N:(h + 1) * N]
                            nc.tensor.matmul(pv[:, sl], vlo, rhs_e,
                                             start=True, stop=False,
                                             skip_group_check=True)
                            nc.tensor.matmul(pv[:, sl], vhi, rhs_o,
                                             start=False, stop=True,
                                             skip_group_check=True)
                    pvv = pv[:].rearrange("n (q h p) -> n q h p", q=2, h=HGB)
                    dvr = Sstv[0:N, b, 1:SCOL, h0:h0 + HGB] \
                        .rearrange("n p h -> n h p")
                    nc.scalar.activation(dvr, pvv[:, 0], AF.Copy)
                    dvi = Simv[:, b, :, h0:h0 + HGB].rearrange("n p h -> n h p")
                    nc.scalar.activation(dvi, pvv[:, 1], AF.Copy)

                # ---- hierarchical scan over 64 pairs ----
                for r in range(1, G1):
                    cfma(sreV[:, :, r, :], simV[:, :, r, :],
                         mre[:, None, :].broadcast_to([N, NG, H]),
                         mim[:, None, :].broadcast_to([N, NG, H]),
                         sreV[:, :, r - 1, :], simV[:, :, r - 1, :],
                         (NG, H))
                for g in range(1, NG):
                    cfma(sreV[:, g, G1 - 1, :][:, None, :],
                         simV[:, g, G1 - 1, :][:, None, :],
                         nre[:, None, :].broadcast_to([N, 1, H]),
                         nim[:, None, :].broadcast_to([N, 1, H]),
                         sreV[:, g - 1, G1 - 1, :][:, None, :],
                         simV[:, g - 1, G1 - 1, :][:, None, :],
                         (1, H))
                for r in range(G1 - 1):
                    cfma(sreV[:, 1:NG, r, :], simV[:, 1:NG, r, :],
                         cbv[:, 0, r][:, None, :].broadcast_to([N, NG - 1, H]),
                         cbv[:, 1, r][:, None, :].broadcast_to([N, NG - 1, H]),
                         sreV[:, 0:NG - 1, G1 - 1, :],
                         simV[:, 0:NG - 1, G1 - 1, :],
                         (NG - 1, H))

                # ---- stack im half across lanes (SBUF->SBUF DMA) ----
                nc.sync.dma_start(Sstv[N:2 * N, b, 1:SCOL, :], Simv[:, b, :, :])

                # ---- phase D: conv + carry + FiLM + residual + store ----
                for hg in range(H // HG):
                    h0 = hg * HG
                    yt = ytp.tile([C2, HG * 2 * T], BF16, tag="yt")
                    rx = rxp.tile([C2, HG * 2 * T], BF16, tag="rx")
                    nc.sync.dma_start(
                        rx[:].rearrange("p (h f) -> p h f", h=HG),
                        xrt.ap()[b, :, h0:h0 + HG, :])
                    for hh in range(HG):
                        h = h0 + hh
                        pdt = ps_d.tile([128, 2 * T], FP32, tag="pd")
                        pd = pdt[0:C2, :]
                        lhs_e = uq[:, h, b, :, 0]
                        lhs_o = uq[:, h, b, :, 1]
                        kth = kt_sb[:, h * T:(h + 1) * T]
                        kt2h = kt2_sb[:, h * T:(h + 1) * T]
                        psth = pst_sb[:, h * T:(h + 1) * T]
                        pmth = pmt_sb[:, h * T:(h + 1) * T]
                        ssth = Sstv[:, b, 0:C2, h]     # [2n, 64] shifted
                        nc.tensor.matmul(pd[:, 0:T], lhs_e, kth,
                                         start=True, stop=False,
                                         skip_group_check=True)
                        nc.tensor.matmul(pd[:, 0:T], ssth, psth,
                                         start=False, stop=True,
                                         skip_group_check=True)
                        nc.tensor.matmul(pd[:, T:2 * T], lhs_o, kth,
                                         start=True, stop=False,
                                         skip_group_check=True)
                        nc.tensor.matmul(pd[:, T:2 * T], lhs_e, kt2h,
                                         start=False, stop=False,
                                         skip_group_check=True)
                        nc.tensor.matmul(pd[:, T:2 * T], ssth, pmth,
                                         start=False, stop=True,
                                         skip_group_check=True)
                        nc.scalar.activation(
                            yt[:, hh * 2 * T:(hh + 1) * 2 * T], pd[:],
                            AF.Gelu,
                            bias=gbB[:, (H + h) * BLOC + b:(H + h) * BLOC + b + 1],
                            scale=gbB[:, h * BLOC + b:h * BLOC + b + 1])
                    eng = nc.vector if hg % 2 == 0 else nc.gpsimd
                    eng.tensor_add(yt[:], yt[:], rx[:])
                    nc.sync.dma_start(
                        y_out.ap()[b].rearrange(
                            "h (p f) -> p h f", f=2 * T)[:, h0:h0 + HG, :],
                        yt[:].rearrange("p (h f) -> p h f", h=HG))

    _split_tail_drain_waits(nc)
    return nc


def kernel(**inputs):
    import ml_dtypes
    if "k" not in _CACHE:
        _CACHE["k"] = _build()
    nc = _CACHE["k"]

    hp = _host_params(
        inputs["log_dt"], inputs["log_A_real"], inputs["A_imag"],
        inputs["C_re"], inputs["C_im"], inputs["D"],
        inputs["W_lin"], inputs["b_lin"])

    x = np.asarray(inputs["x"], dtype=np.float32)
    res_w = np.asarray(inputs["res_w"], dtype=np.float32)
    cond = np.ascontiguousarray(
        np.asarray(inputs["conditional_information"], dtype=np.float32))
    film_W = np.asarray(inputs["film_W"], dtype=np.float32)
    film_b = np.asarray(inputs["film_b"], dtype=np.float32)

    bf16 = ml_dtypes.bfloat16
    xb = x.astype(bf16)                                       # [B,H,L]
    ones_row = np.ones((B, 1, L), dtype=bf16)
    xbf = np.ascontiguousarray(np.concatenate([xb, ones_row], axis=1))
    rx = (x * res_w[None, :, None]).astype(bf16)
    xrt = np.ascontiguousarray(
        rx.reshape(B, H, C2, 2 * T).transpose(0, 2, 1, 3))

    common = dict(hp)
    common["film_WT"] = np.ascontiguousarray(film_W.T)
    common["film_bl"] = np.ascontiguousarray(film_b[None, :])
    common["ones1"] = np.ones((1, BLOC), np.float32)

    in_maps = []
    for c_ in range(NCORES):
        m = dict(common)
        m["xbf"] = np.ascontiguousarray(xbf[c_ * BLOC:(c_ + 1) * BLOC])
        m["xrt"] = np.ascontiguousarray(xrt[c_ * BLOC:(c_ + 1) * BLOC])
        m["condT"] = np.ascontiguousarray(cond[c_ * BLOC:(c_ + 1) * BLOC].T)
        in_maps.append(m)

    res = run_bass_kernel_spmd(nc, in_maps, core_ids=list(range(NCORES)))
    out = np.concatenate(
        [np.asarray(res.results[c_]["y_out"]) for c_ in range(NCORES)], axis=0)
    return out.astype(np.float32)


# revision 22
# speedup vs baseline: 2.5072x; 1.0780x over previous
"""Trainium2 Bass kernel for nn_Block_19301583028789 (v2).

Pipeline per batch: channel Linear -> erf-GELU -> S4D (chunked linear
recurrence, exact) -> FiLM -> erf-GELU -> per-channel residual.

v2 redesign vs v1 (587us):
- all matmuls bf16 (1 cyc/row), x pre-cast to bf16 host-side
- W=2 pair-level state scan: 64 scanned states instead of 128 (pair
  summaries built by accumulating matmuls at no extra PE cost)
- [c,t]-form conv output (out[c,t] = u^T KT + S^T P), chunk parity in the
  free dim -> no PE transposes, no extra PSUM->SBUF copy passes
- re/im carry contraction stacked on 128 partitions (1 matmul, not 2);
  im half moved across lanes by one SBUF->SBUF DMA after the scan
- few, large DMAs (HWDGE fixed cost is ~625ns per DMA instruction)
- residual pre-scaled by res_w host-side; bf16 output, host upcasts

Sharding: data-parallel over batch B=16 across 8 cores (2 per core).
"""

import numpy as np

import concourse.bass as bass
import concourse.tile as tile
import concourse.mybir as mybir
from concourse.bass_utils import run_bass_kernel_spmd

B, H, L = 16, 64, 16384
N, CD = 64, 32
T = 128
C = L // T            # 128 chunks
C2 = C // 2           # 64 chunk pairs (scan granularity)
G1 = 8                # pairs per scan group
NG = C2 // G1         # 8 groups
NCORES = 8
BLOC = B // NCORES    # 2
SCOL = C2 + 1         # pad column + 64 pair states
FP32 = mybir.dt.float32
BF16 = mybir.dt.bfloat16
AF = mybir.ActivationFunctionType

_CACHE = {}


def _split_tail_drain_waits(nc, max_waits=1):
    """Walrus TPB_CTRL lowering only accepts 1 sync-wait per Drain/NoOp."""
    for fn in nc.m.functions:
        for blk in fn.blocks:
            insts = blk.instructions
            i = 0
            while i < len(insts):
                inst = insts[i]
                si = inst.sync_info
                if si is not None and len(si.on_wait) > max_waits:
                    extra = list(si.on_wait[:-max_waits])
                    keep = list(si.on_wait[-max_waits:])
                    nops = [
                        mybir.InstNoOp(
                            name=f"{inst.name}-waitsplit{k}",
                            opcode="NoOp",
                            engine=inst.engine,
                            sync_info=mybir.SyncInfo(on_wait=[w], on_update=[]),
                        )
                        for k, w in enumerate(extra)
                    ]
                    si.on_wait = keep
                    for k, nop in enumerate(nops):
                        insts.insert(i + k, nop)
                    i += len(nops)
                i += 1


def _host_params(log_dt, log_A_real, A_imag, C_re, C_im, D, W_lin, b_lin):
    """Parameter-derived constant matrices (fp64 host math), bf16-packed."""
    import ml_dtypes
    bf = lambda a: np.ascontiguousarray(
        np.asarray(a, dtype=np.float64).astype(np.float32).astype(ml_dtypes.bfloat16))

    dt = np.exp(log_dt.astype(np.float64))[:, None]            # [H,1]
    A = -np.exp(log_A_real.astype(np.float64)) + 1j * A_imag.astype(np.float64)
    dtA = A * dt                                               # [H,N]
    coef = (C_re.astype(np.float64) + 1j * C_im.astype(np.float64)) \
        * (np.exp(dtA) - 1.0) / A                              # [H,N]

    ks = np.arange(2 * T + 2)
    lp = np.exp(dtA[:, :, None] * ks[None, None, :])           # [H,N,2T+2]

    # local-conv kernel, D folded into tap 0
    K = 2.0 * np.real(np.einsum("hn,hnm->hm", coef, lp[:, :, :T]))  # [H,T]
    K[:, 0] += D.astype(np.float64)
    # KT[t', h, t] = K_h[t - t'] for t >= t'   (rhs of out[c,t] local conv)
    idx = np.arange(T)
    dmat = idx[None, :] - idx[:, None]                         # [t', t]
    KT = np.where(dmat[None] >= 0, K[:, np.clip(dmat, 0, T - 1)], 0.0)  # [H,t',t]
    KT = np.transpose(KT, (1, 0, 2))                           # [t',H,t]
    # KT2: kernel taps T..2T-1 = carry from the immediately-preceding chunk
    # (K2[d] = 2 Re sum_n coef lam^d, d = T + t - t', dense)
    K2 = 2.0 * np.real(np.einsum("hn,hnm->hm", coef, lp[:, :, :2 * T]))  # [H,2T]
    KT2 = K2[:, T + dmat]                                      # [H,t',t]
    KT2 = np.transpose(KT2, (1, 0, 2))

    # pair-summary lhsT tables (V[p] = sum over 2 chunks of lam^(2T-1-tau) u)
    VA_lo = np.transpose(lp[:, :, ::-1][:, :, 2:T + 2], (2, 0, 1))      # lam^(2T-1-t) [t,H,N]
    VA_hi = np.transpose(lp[:, :, ::-1][:, :, T + 2:2 * T + 2], (2, 0, 1))  # lam^(T-1-t)

    # carry rhs, stacked (re | -im): y[c,t] += Re(sum_n S[n] P[n,t])
    P = 2.0 * coef[:, :, None] * lp[:, :, 1:T + 1]             # [H,N,T]
    mu = lp[:, :, T]                                           # lam^T [H,N]
    Pmu = P * mu[:, :, None]
    PST = np.concatenate([P.real, -P.imag], axis=1)            # [H,2N,T]
    PMT = np.concatenate([Pmu.real, -Pmu.imag], axis=1)
    PST = np.transpose(PST, (1, 0, 2))                         # [2N,H,T]
    PMT = np.transpose(PMT, (1, 0, 2))

    # scan coefficients over pairs: ratio nu = mu^2 = lam^(2T)
    nu = lp[:, :, 2 * T]                                       # [H,N]
    nuP = nu[None, :, :] ** (1 + np.arange(G1)[:, None, None]) # [G1,H,N]
    MU2_re = nu.real.T; MU2_im = nu.imag.T                     # [N,H]
    NU2_re = nuP[G1 - 1].real.T; NU2_im = nuP[G1 - 1].imag.T   # nu^G1
    CB_re = np.transpose(nuP[:G1 - 1].real, (2, 0, 1))         # [N,G1-1,H]
    CB_im = np.transpose(nuP[:G1 - 1].imag, (2, 0, 1))

    WB = np.concatenate([W_lin.T.astype(np.float64),
                         b_lin.astype(np.float64)[None, :]], 0)  # [H+1,H]

    return {
        "WB": bf(WB), "KT": bf(KT), "KT2": bf(KT2),
        "VA_lo_re": bf(VA_lo.real), "VA_lo_im": bf(VA_lo.imag),
        "VA_hi_re": bf(VA_hi.real), "VA_hi_im": bf(VA_hi.imag),
        "PST": bf(PST), "PMT": bf(PMT),
        "MU2_re": bf(MU2_re), "MU2_im": bf(MU2_im),
        "NU2_re": bf(NU2_re), "NU2_im": bf(NU2_im),
        "CB_re": bf(CB_re), "CB_im": bf(CB_im),
    }


def _build():
    nc = bass.Bass("TRN2", target_bir_lowering=False, debug=False)

    def din(name, shape, dtype=BF16):
        return nc.dram_tensor(name, list(shape), dtype, kind="ExternalInput")

    xbf = din("xbf", [BLOC, H + 1, L])           # x bf16 + ones row (phase A)
    # res_w*x in the h-paired store layout: [b, (half c2), hp, 256] bf16
    xrt = din("xrt", [BLOC, 2 * C2, H // 2, 2 * T])
    WB = din("WB", [H + 1, H])
    KT = din("KT", [T, H, T])
    KT2 = din("KT2", [T, H, T])
    VA_lo_re = din("VA_lo_re", [T, H, N]); VA_lo_im = din("VA_lo_im", [T, H, N])
    VA_hi_re = din("VA_hi_re", [T, H, N]); VA_hi_im = din("VA_hi_im", [T, H, N])
    PST = din("PST", [2 * N, H, T]); PMT = din("PMT", [2 * N, H, T])
    MU2_re = din("MU2_re", [N, H]); MU2_im = din("MU2_im", [N, H])
    NU2_re = din("NU2_re", [N, H]); NU2_im = din("NU2_im", [N, H])
    CB_re = din("CB_re", [N, G1 - 1, H]); CB_im = din("CB_im", [N, G1 - 1, H])
    film_WT = din("film_WT", [CD, 2 * H], FP32)
    film_bl = din("film_bl", [1, 2 * H], FP32)
    condT = din("condT", [CD, BLOC], FP32)
    ones1 = din("ones1", [1, BLOC], FP32)
    gb_scratch = nc.dram_tensor("gb_scratch", [2 * H * BLOC], FP32)
    y_out = nc.dram_tensor("y_out", [BLOC, H, L], BF16, kind="ExternalOutput")

    HGB = 4                      # h batch for phase B psum
    HG = 8                       # h batch for phase D store
    with tile.TileContext(nc) as tc:
        with (
            tc.tile_pool(name="big", bufs=1) as big,
            tc.tile_pool(name="cpar", bufs=1) as cpar,
            tc.tile_pool(name="xa", bufs=3) as xa,
            tc.tile_pool(name="tmp", bufs=1) as tmp,
            tc.tile_pool(name="yt", bufs=2) as ytp,
            tc.tile_pool(name="rx", bufs=2) as rxp,
            tc.tile_pool(name="ps_a", bufs=2, space="PSUM") as ps_a,
            tc.tile_pool(name="ps_b", bufs=2, space="PSUM") as ps_b,
            tc.tile_pool(name="ps_d", bufs=4, space="PSUM") as ps_d,
        ):
            # ---------------- resident tensors ----------------
            u = big.tile([T, H * BLOC * C], BF16, tag="u")      # [t,(h,b,c)]
            uv = u[:].rearrange("t (h b c) -> t h b c", h=H, b=BLOC)
            uq = u[:].rearrange("t (h b p q) -> t h b p q", h=H, b=BLOC, q=2)
            # stacked carry lhsT: rows 0:64 S_re, 64:128 S_im; cols (b, pad+p, h)
            Sst = big.tile([2 * N, BLOC * SCOL * H], BF16, tag="Sst")
            Sstv = Sst[:].rearrange("n (b p h) -> n b p h", b=BLOC, p=SCOL)
            # im-scratch on lanes 0:64 (scan runs here; DMA'd to Sst[64:128])
            Sim = big.tile([N, BLOC * C2 * H], BF16, tag="Sim")
            Simv = Sim[:].rearrange("n (b p h) -> n b p h", b=BLOC, p=C2)

            for b in range(BLOC):   # zero pad columns (both halves)
                nc.vector.memset(Sstv[:, b, 0, :], 0.0)

            # ---------------- parameters ----------------
            # x-path params on the SP queue (needed first); big D-phase
            # params issued later on the Pool/SWDGE queue (bypasses HWDGE).
            wb_sb = cpar.tile([H + 1, H], BF16, tag="wb")
            nc.sync.dma_start(wb_sb[:], WB.ap())

            # FiLM prologue -> per-(h,b) scalar columns
            fwt = cpar.tile([CD, 2 * H], FP32, tag="fwt")
            nc.gpsimd.dma_start(fwt[:], film_WT.ap())
            fbl = cpar.tile([1, 2 * H], FP32, tag="fbl")
            nc.gpsimd.dma_start(fbl[:], film_bl.ap())
            ct = cpar.tile([CD, BLOC], FP32, tag="ct")
            nc.gpsimd.dma_start(ct[:], condT.ap())
            on1 = cpar.tile([1, BLOC], FP32, tag="on1")
            nc.gpsimd.dma_start(on1[:], ones1.ap())
            gps_t = ps_d.tile([128, 2 * T], FP32, tag="pd")
            gps = gps_t[0:2 * H, 0:BLOC]
            nc.tensor.matmul(gps, fwt[:], ct[:], start=True, stop=False)
            nc.tensor.matmul(gps, fbl[:], on1[:], start=False, stop=True)
            gb_sb = cpar.tile([2 * H, BLOC], FP32, tag="gb")
            nc.scalar.copy(gb_sb[:], gps)
            nc.gpsimd.dma_start(
                gb_scratch.ap().rearrange("(r b) -> r b", b=BLOC), gb_sb[:])
            # paired broadcast columns: col (q, hp, b) rows 0:64 = gb[q,2hp,b],
            # rows 64:128 = gb[q,2hp+1,b]
            gbv = gb_scratch.ap().rearrange(
                "(q hp e b) -> q hp e b", q=2, hp=H // 2, e=2)
            gbP = cpar.tile([128, 2 * (H // 2) * BLOC], FP32, tag="gbP")
            gbPv = gbP[:].rearrange("p (q hp b) -> p q hp b", q=2, hp=H // 2)
            nc.gpsimd.dma_start(
                gbPv[0:N], gbv[:, :, 0, :][None].broadcast_to(
                    [N, 2, H // 2, BLOC]))
            nc.gpsimd.dma_start(
                gbPv[N:128], gbv[:, :, 1, :][None].broadcast_to(
                    [N, 2, H // 2, BLOC]))

            # ---------------- phase A: u = gelu(W x + b) ----------------
            def phase_a(b):
                XC = 8      # chunks per x DMA; 4 chunks per psum tile
                for cg in range(C // XC):
                    xt = xa.tile([H + 1, XC * T], BF16, tag="xt")
                    nc.sync.dma_start(
                        xt[:], xbf.ap()[b, :, cg * XC * T:(cg + 1) * XC * T])
                    for half in range(2):
                        pa = ps_a.tile([T, 4 * H], FP32)
                        for cc in range(4):
                            cx = half * 4 + cc
                            nc.tensor.matmul(
                                pa[:, cc * H:(cc + 1) * H],
                                xt[:, cx * T:(cx + 1) * T], wb_sb[:],
                                start=True, stop=True, skip_group_check=True)
                        c0 = cg * XC + half * 4
                        dst = uv[:, :, b, c0:c0 + 4].rearrange("t h c -> t c h")
                        nc.scalar.activation(dst, pa[:], AF.Gelu)

            phase_a(0)

            # B-phase params (needed right after A(b=0)): Pool queue
            va_sb = {}
            for nm, tens in (("lr", VA_lo_re), ("hr", VA_hi_re),
                             ("li", VA_lo_im), ("hi", VA_hi_im)):
                t_ = cpar.tile([T, H * N], BF16, tag="va" + nm)
                nc.gpsimd.dma_start(t_[:].rearrange("t (h f) -> t h f", h=H),
                                    tens.ap())
                va_sb[nm] = t_
            mu2 = cpar.tile([N, 2 * H], BF16, tag="mu2")
            nc.gpsimd.dma_start(mu2[:, 0:H], MU2_re.ap())
            nc.gpsimd.dma_start(mu2[:, H:], MU2_im.ap())
            nu2 = cpar.tile([N, 2 * H], BF16, tag="nu2")
            nc.gpsimd.dma_start(nu2[:, 0:H], NU2_re.ap())
            nc.gpsimd.dma_start(nu2[:, H:], NU2_im.ap())
            cb = cpar.tile([N, 2 * (G1 - 1) * H], BF16, tag="cb")
            cbv = cb[:].rearrange("n (q r h) -> n q r h", q=2, r=G1 - 1)
            nc.gpsimd.dma_start(cbv[:, 0], CB_re.ap())
            nc.gpsimd.dma_start(cbv[:, 1], CB_im.ap())

            phase_a(1)

            # D-phase params (needed ~after the first scan): Pool queue
            kt_sb = cpar.tile([T, H * T], BF16, tag="kt")
            nc.gpsimd.dma_start(kt_sb[:].rearrange("t (h f) -> t h f", h=H),
                                KT.ap())
            kt2_sb = cpar.tile([T, H * T], BF16, tag="kt2")
            nc.gpsimd.dma_start(kt2_sb[:].rearrange("t (h f) -> t h f", h=H),
                                KT2.ap())
            pst_sb = cpar.tile([2 * N, H * T], BF16, tag="pst")
            nc.gpsimd.dma_start(pst_sb[:].rearrange("n (h f) -> n h f", h=H),
                                PST.ap())
            pmt_sb = cpar.tile([2 * N, H * T], BF16, tag="pmt")
            nc.gpsimd.dma_start(pmt_sb[:].rearrange("n (h f) -> n h f", h=H),
                                PMT.ap())

            # ---------------- per-b: phase B + scan + phase D ----------------
            mre = mu2[:, 0:H]; mim = mu2[:, H:]
            nre = nu2[:, 0:H]; nim = nu2[:, H:]

            def cfma(dre, dim_, wre, wim, sre, sim_, fshape):
                """d += w * s (complex); w broadcast tiles, all bf16."""
                nf = int(np.prod(fshape))
                t1 = tmp.tile([N, NG * H], BF16, tag="t1")
                t2 = tmp.tile([N, NG * H], BF16, tag="t2")
                a = t1[:, :nf].rearrange("n (x y) -> n x y", x=fshape[0])
                bq = t2[:, :nf].rearrange("n (x y) -> n x y", x=fshape[0])
                nc.vector.tensor_mul(a, wre, sre)
                nc.vector.tensor_mul(bq, wim, sim_)
                nc.vector.tensor_sub(a, a, bq)
                nc.vector.tensor_add(dre, dre, a)
                nc.vector.tensor_mul(a, wre, sim_)
                nc.vector.tensor_mul(bq, wim, sre)
                nc.vector.tensor_add(a, a, bq)
                nc.vector.tensor_add(dim_, dim_, a)

            for b in range(BLOC):
                # scan views [n, g, r, h]
                sreV = Sstv[0:N, b, 1:SCOL, :].rearrange(
                    "n (g r) h -> n g r h", g=NG)
                simV = Simv[:, b, :, :].rearrange("n (g r) h -> n g r h", g=NG)

                # ---- phase B: V pair-summaries ----
                for hg in range(H // HGB):
                    h0 = hg * HGB
                    pv = ps_b.tile([N, 2 * HGB * C2], FP32)   # (q, h, p)
                    for hh in range(HGB):
                        h = h0 + hh
                        rhs_e = uq[:, h, b, :, 0]
                        rhs_o = uq[:, h, b, :, 1]
                        for q, (lo, hi) in enumerate(
                                (("lr", "hr"), ("li", "hi"))):
                            sl = slice((q * HGB + hh) * C2,
                                       (q * HGB + hh + 1) * C2)
                            vlo = va_sb[lo][:, h * N:(h + 1) * N]
                            vhi = va_sb[hi][:, h * N:(h + 1) * N]
                            nc.tensor.matmul(pv[:, sl], vlo, rhs_e,
                                             start=True, stop=False,
                                             skip_group_check=True)
                            nc.tensor.matmul(pv[:, sl], vhi, rhs_o,
                                             start=False, stop=True,
                                             skip_group_check=True)
                    pvv = pv[:].rearrange("n (q h p) -> n q h p", q=2, h=HGB)
                    dvr = Sstv[0:N, b, 1:SCOL, h0:h0 + HGB] \
                        .rearrange("n p h -> n h p")
                    nc.scalar.activation(dvr, pvv[:, 0], AF.Copy)
                    dvi = Simv[:, b, :, h0:h0 + HGB].rearrange("n p h -> n h p")
                    nc.scalar.activation(dvi, pvv[:, 1], AF.Copy)

                # ---- hierarchical scan over 64 pairs ----
                for r in range(1, G1):
                    cfma(sreV[:, :, r, :], simV[:, :, r, :],
                         mre[:, None, :].broadcast_to([N, NG, H]),
                         mim[:, None, :].broadcast_to([N, NG, H]),
                         sreV[:, :, r - 1, :], simV[:, :, r - 1, :],
                         (NG, H))
                for g in range(1, NG):
                    cfma(sreV[:, g, G1 - 1, :][:, None, :],
                         simV[:, g, G1 - 1, :][:, None, :],
                         nre[:, None, :].broadcast_to([N, 1, H]),
                         nim[:, None, :].broadcast_to([N, 1, H]),
                         sreV[:, g - 1, G1 - 1, :][:, None, :],
                         simV[:, g - 1, G1 - 1, :][:, None, :],
                         (1, H))
                for r in range(G1 - 1):
                    cfma(sreV[:, 1:NG, r, :], simV[:, 1:NG, r, :],
                         cbv[:, 0, r][:, None, :].broadcast_to([N, NG - 1, H]),
                         cbv[:, 1, r][:, None, :].broadcast_to([N, NG - 1, H]),
                         sreV[:, 0:NG - 1, G1 - 1, :],
                         simV[:, 0:NG - 1, G1 - 1, :],
                         (NG - 1, H))

                # ---- stack im half across lanes (SBUF->SBUF DMA) ----
                nc.sync.dma_start(Sstv[N:2 * N, b, 1:SCOL, :], Simv[:, b, :, :])

                # ---- phase D: conv + carry + FiLM + residual + store ----
                # h-pairs share one [128, 256] psum tile: even h rows 0:64,
                # odd h rows 64:128; one act per pair (gbP columns)
                HP = HG // 2     # 4 pairs per store group
                for hg in range(H // HG):
                    hp0 = hg * HP
                    yt = ytp.tile([128, HP * 2 * T], BF16, tag="yt")
                    rx = rxp.tile([128, HP * 2 * T], BF16, tag="rx")
                    nc.sync.dma_start(
                        rx[:].rearrange("p (h f) -> p h f", h=HP),
                        xrt.ap()[b, :, hp0:hp0 + HP, :])
                    for hh in range(HP):
                        hp = hp0 + hh
                        pdt = ps_d.tile([128, 2 * T], FP32, tag="pd")
                        for e in range(2):
                            h = 2 * hp + e
                            pd = pdt[64 * e:64 * (e + 1), :]
                            lhs_e = uq[:, h, b, :, 0]
                            lhs_o = uq[:, h, b, :, 1]
                            kth = kt_sb[:, h * T:(h + 1) * T]
                            kt2h = kt2_sb[:, h * T:(h + 1) * T]
                            psth = pst_sb[:, h * T:(h + 1) * T]
                            pmth = pmt_sb[:, h * T:(h + 1) * T]
                            ssth = Sstv[:, b, 0:C2, h]   # [2n, 64] shifted
                            nc.tensor.matmul(pd[:, 0:T], lhs_e, kth,
                                             start=True, stop=False,
                                             skip_group_check=True)
                            nc.tensor.matmul(pd[:, 0:T], ssth, psth,
                                             start=False, stop=True,
                                             skip_group_check=True)
                            nc.tensor.matmul(pd[:, T:2 * T], lhs_o, kth,
                                             start=True, stop=False,
                                             skip_group_check=True)
                            nc.tensor.matmul(pd[:, T:2 * T], lhs_e, kt2h,
                                             start=False, stop=False,
                                             skip_group_check=True)
                            nc.tensor.matmul(pd[:, T:2 * T], ssth, pmth,
                                             start=False, stop=True,
                                             skip_group_check=True)
                        sc = (0 * (H // 2) + hp) * BLOC + b
                        bc = (1 * (H // 2) + hp) * BLOC + b
                        nc.scalar.activation(
                            yt[:, hh * 2 * T:(hh + 1) * 2 * T], pdt[:],
                            AF.Gelu,
                            bias=gbP[:, bc:bc + 1], scale=gbP[:, sc:sc + 1])
                    eng = nc.gpsimd if b == 0 else nc.vector
                    eng.tensor_add(yt[:], yt[:], rx[:])
                    nc.sync.dma_start(
                        y_out.ap()[b].rearrange(
                            "(hp e) (p f) -> e p hp f", e=2, f=2 * T)
                        [:, :, hp0:hp0 + HP, :],
                        yt[:].rearrange("p (h f) -> p h f", h=HP))

    _split_tail_drain_waits(nc)
    return nc


def kernel(**inputs):
    import ml_dtypes
    if "k" not in _CACHE:
        _CACHE["k"] = _build()
    nc = _CACHE["k"]

    hp = _host_params(
        inputs["log_dt"], inputs["log_A_real"], inputs["A_imag"],
        inputs["C_re"], inputs["C_im"], inputs["D"],
        inputs["W_lin"], inputs["b_lin"])

    x = np.asarray(inputs["x"], dtype=np.float32)
    res_w = np.asarray(inputs["res_w"], dtype=np.float32)
    cond = np.ascontiguousarray(
        np.asarray(inputs["conditional_information"], dtype=np.float32))
    film_W = np.asarray(inputs["film_W"], dtype=np.float32)
    film_b = np.asarray(inputs["film_b"], dtype=np.float32)

    bf16 = ml_dtypes.bfloat16
    xb = x.astype(bf16)                                       # [B,H,L]
    ones_row = np.ones((B, 1, L), dtype=bf16)
    xbf = np.ascontiguousarray(np.concatenate([xb, ones_row], axis=1))
    rx = (x * res_w[None, :, None]).astype(bf16)
    # [b, hp, e, c2, f] -> [b, (e c2), hp, f]
    xrt = np.ascontiguousarray(
        rx.reshape(B, H // 2, 2, C2, 2 * T).transpose(0, 2, 3, 1, 4)
        .reshape(B, 2 * C2, H // 2, 2 * T))

    common = dict(hp)
    common["film_WT"] = np.ascontiguousarray(film_W.T)
    common["film_bl"] = np.ascontiguousarray(film_b[None, :])
    common["ones1"] = np.ones((1, BLOC), np.float32)

    in_maps = []
    for c_ in range(NCORES):
        m = dict(common)
        m["xbf"] = np.ascontiguousarray(xbf[c_ * BLOC:(c_ + 1) * BLOC])
        m["xrt"] = np.ascontiguousarray(xrt[c_ * BLOC:(c_ + 1) * BLOC])
        m["condT"] = np.ascontiguousarray(cond[c_ * BLOC:(c_ + 1) * BLOC].T)
        in_maps.append(m)

    res = run_bass_kernel_spmd(nc, in_maps, core_ids=list(range(NCORES)))
    out = np.concatenate(
        [np.asarray(res.results[c_]["y_out"]) for c_ in range(NCORES)], axis=0)
    return out.astype(np.float32)


# revision 29
# speedup vs baseline: 3.2140x; 1.2819x over previous
"""Trainium2 Bass kernel for nn_Block_19301583028789 (v2).

Pipeline per batch: channel Linear -> erf-GELU -> S4D (chunked linear
recurrence, exact) -> FiLM -> erf-GELU -> per-channel residual.

v2 redesign vs v1 (587us):
- all matmuls bf16 (1 cyc/row), x pre-cast to bf16 host-side
- W=2 pair-level state scan: 64 scanned states instead of 128 (pair
  summaries built by accumulating matmuls at no extra PE cost)
- [c,t]-form conv output (out[c,t] = u^T KT + S^T P), chunk parity in the
  free dim -> no PE transposes, no extra PSUM->SBUF copy passes
- re/im carry contraction stacked on 128 partitions (1 matmul, not 2);
  im half moved across lanes by one SBUF->SBUF DMA after the scan
- few, large DMAs (HWDGE fixed cost is ~625ns per DMA instruction)
- residual pre-scaled by res_w host-side; bf16 output, host upcasts

Sharding: data-parallel over batch B=16 across 8 cores (2 per core).
"""

import numpy as np

import concourse.bass as bass
import concourse.tile as tile
import concourse.mybir as mybir
from concourse.bass_utils import run_bass_kernel_spmd

B, H, L = 16, 64, 16384
N, CD = 64, 32
T = 128
C = L // T            # 128 chunks
C2 = C // 2           # 64 chunk pairs (scan granularity)
G1 = 8                # pairs per scan group
NG = C2 // G1         # 8 groups
NCORES = 8
BLOC = B // NCORES    # 2
SCOL = C2 + 1         # pad column + 64 pair states
FP32 = mybir.dt.float32
BF16 = mybir.dt.bfloat16
AF = mybir.ActivationFunctionType

_CACHE = {}


def _split_tail_drain_waits(nc, max_waits=1):
    """Walrus TPB_CTRL lowering only accepts 1 sync-wait per Drain/NoOp."""
    for fn in nc.m.functions:
        for blk in fn.blocks:
            insts = blk.instructions
            i = 0
            while i < len(insts):
                inst = insts[i]
                si = inst.sync_info
                if si is not None and len(si.on_wait) > max_waits:
                    extra = list(si.on_wait[:-max_waits])
                    keep = list(si.on_wait[-max_waits:])
                    nops = [
                        mybir.InstNoOp(
                            name=f"{inst.name}-waitsplit{k}",
                            opcode="NoOp",
                            engine=inst.engine,
                            sync_info=mybir.SyncInfo(on_wait=[w], on_update=[]),
                        )
                        for k, w in enumerate(extra)
                    ]
                    si.on_wait = keep
                    for k, nop in enumerate(nops):
                        insts.insert(i + k, nop)
                    i += len(nops)
                i += 1


def _host_params(log_dt, log_A_real, A_imag, C_re, C_im, D, W_lin, b_lin):
    """Parameter-derived constant matrices (fp64 host math), bf16-packed."""
    import ml_dtypes
    bf = lambda a: np.ascontiguousarray(
        np.asarray(a, dtype=np.float64).astype(np.float32).astype(ml_dtypes.bfloat16))

    dt = np.exp(log_dt.astype(np.float64))[:, None]            # [H,1]
    A = -np.exp(log_A_real.astype(np.float64)) + 1j * A_imag.astype(np.float64)
    dtA = A * dt                                               # [H,N]
    coef = (C_re.astype(np.float64) + 1j * C_im.astype(np.float64)) \
        * (np.exp(dtA) - 1.0) / A                              # [H,N]

    ks = np.arange(2 * T + 2)
    lp = np.exp(dtA[:, :, None] * ks[None, None, :])           # [H,N,2T+2]

    # local-conv kernel, D folded into tap 0
    K = 2.0 * np.real(np.einsum("hn,hnm->hm", coef, lp[:, :, :T]))  # [H,T]
    K[:, 0] += D.astype(np.float64)
    # KT[t', h, t] = K_h[t - t'] for t >= t'   (rhs of out[c,t] local conv)
    idx = np.arange(T)
    dmat = idx[None, :] - idx[:, None]                         # [t', t]
    KT = np.where(dmat[None] >= 0, K[:, np.clip(dmat, 0, T - 1)], 0.0)  # [H,t',t]
    KT = np.transpose(KT, (1, 0, 2))                           # [t',H,t]
    # KT2: kernel taps T..2T-1 = carry from the immediately-preceding chunk
    # (K2[d] = 2 Re sum_n coef lam^d, d = T + t - t', dense)
    K2 = 2.0 * np.real(np.einsum("hn,hnm->hm", coef, lp[:, :, :2 * T]))  # [H,2T]
    KT2 = K2[:, T + dmat]                                      # [H,t',t]
    KT2 = np.transpose(KT2, (1, 0, 2))

    # pair-summary lhsT tables (V[p] = sum over 2 chunks of lam^(2T-1-tau) u)
    VA_lo = np.transpose(lp[:, :, ::-1][:, :, 2:T + 2], (2, 0, 1))      # lam^(2T-1-t) [t,H,N]
    VA_hi = np.transpose(lp[:, :, ::-1][:, :, T + 2:2 * T + 2], (2, 0, 1))  # lam^(T-1-t)

    # carry rhs, stacked (re | -im): y[c,t] += Re(sum_n S[n] P[n,t])
    P = 2.0 * coef[:, :, None] * lp[:, :, 1:T + 1]             # [H,N,T]
    mu = lp[:, :, T]                                           # lam^T [H,N]
    Pmu = P * mu[:, :, None]
    PST = np.concatenate([P.real, -P.imag], axis=1)            # [H,2N,T]
    PMT = np.concatenate([Pmu.real, -Pmu.imag], axis=1)
    PST = np.transpose(PST, (1, 0, 2))                         # [2N,H,T]
    PMT = np.transpose(PMT, (1, 0, 2))

    # scan coefficients over pairs: ratio nu = mu^2 = lam^(2T)
    # lane-packed layout: row = n + 64*(h // 32), col = h % 32
    def lanepack(a_nh):                                        # [N,H] -> [128,32]
        return a_nh.reshape(N, 2, H // 2).transpose(1, 0, 2).reshape(2 * N, H // 2)

    nu = lp[:, :, 2 * T]                                       # [H,N]
    nuP = nu[None, :, :] ** (1 + np.arange(G1)[:, None, None]) # [G1,H,N]
    MU2_re = lanepack(nu.real.T); MU2_im = lanepack(nu.imag.T)
    NU2_re = lanepack(nuP[G1 - 1].real.T); NU2_im = lanepack(nuP[G1 - 1].imag.T)
    CB_re = np.stack([lanepack(nuP[r].real.T) for r in range(G1 - 1)], 1)
    CB_im = np.stack([lanepack(nuP[r].imag.T) for r in range(G1 - 1)], 1)  # [128,G1-1,32]

    WB = np.concatenate([W_lin.T.astype(np.float64),
                         b_lin.astype(np.float64)[None, :]], 0)  # [H+1,H]

    return {
        "WB": bf(WB), "KT": bf(KT), "KT2": bf(KT2),
        "VA_lo_re": bf(VA_lo.real), "VA_lo_im": bf(VA_lo.imag),
        "VA_hi_re": bf(VA_hi.real), "VA_hi_im": bf(VA_hi.imag),
        "PST": bf(PST), "PMT": bf(PMT),
        "MU2_re": bf(MU2_re), "MU2_im": bf(MU2_im),
        "NU2_re": bf(NU2_re), "NU2_im": bf(NU2_im),
        "CB_re": bf(CB_re), "CB_im": bf(CB_im),
    }


def _build():
    nc = bass.Bass("TRN2", target_bir_lowering=False, debug=False)

    def din(name, shape, dtype=BF16):
        return nc.dram_tensor(name, list(shape), dtype, kind="ExternalInput")

    xbf = din("xbf", [BLOC, H + 1, L])           # x bf16 + ones row (phase A)
    # res_w*x in the h-paired store layout: [b, (half c2), hp, 256] bf16
    xrt = din("xrt", [BLOC, 2 * C2, H // 2, 2 * T])
    WB = din("WB", [H + 1, H])
    KT = din("KT", [T, H, T])
    KT2 = din("KT2", [T, H, T])
    VA_lo_re = din("VA_lo_re", [T, H, N]); VA_lo_im = din("VA_lo_im", [T, H, N])
    VA_hi_re = din("VA_hi_re", [T, H, N]); VA_hi_im = din("VA_hi_im", [T, H, N])
    PST = din("PST", [2 * N, H, T]); PMT = din("PMT", [2 * N, H, T])
    HH2 = H // 2
    MU2_re = din("MU2_re", [2 * N, HH2]); MU2_im = din("MU2_im", [2 * N, HH2])
    NU2_re = din("NU2_re", [2 * N, HH2]); NU2_im = din("NU2_im", [2 * N, HH2])
    CB_re = din("CB_re", [2 * N, G1 - 1, HH2])
    CB_im = din("CB_im", [2 * N, G1 - 1, HH2])
    film_WT = din("film_WT", [CD, 2 * H], FP32)
    film_bl = din("film_bl", [1, 2 * H], FP32)
    condT = din("condT", [CD, BLOC], FP32)
    ones1 = din("ones1", [1, BLOC], FP32)
    gb_scratch = nc.dram_tensor("gb_scratch", [2 * H * BLOC], FP32)
    y_out = nc.dram_tensor("y_out", [BLOC, H, L], BF16, kind="ExternalOutput")

    HGB = 4                      # h batch for phase B psum
    HG = 8                       # h batch for phase D store
    with tile.TileContext(nc) as tc:
        with (
            tc.tile_pool(name="big", bufs=1) as big,
            tc.tile_pool(name="cpar", bufs=1) as cpar,
            tc.tile_pool(name="xa", bufs=3) as xa,
            tc.tile_pool(name="tmp", bufs=1) as tmp,
            tc.tile_pool(name="yt", bufs=3) as ytp,
            tc.tile_pool(name="rx", bufs=3) as rxp,
            tc.tile_pool(name="ps_a", bufs=2, space="PSUM") as ps_a,
            tc.tile_pool(name="ps_b", bufs=2, space="PSUM") as ps_b,
            tc.tile_pool(name="ps_d", bufs=4, space="PSUM") as ps_d,
        ):
            # ---------------- resident tensors ----------------
            u = big.tile([T, H * BLOC * C], BF16, tag="u")      # [t,(h,b,c)]
            uv = u[:].rearrange("t (h b c) -> t h b c", h=H, b=BLOC)
            uq = u[:].rearrange("t (h b p q) -> t h b p q", h=H, b=BLOC, q=2)
            # stacked carry lhsT: rows 0:64 S_re, 64:128 S_im;
            # cols (b, hh, pad+p, h32) with h = hh*32 + h32
            Sst = big.tile([2 * N, BLOC * 2 * SCOL * HH2], BF16, tag="Sst")
            Sstv = Sst[:].rearrange("n (b e p h) -> n b e p h", b=BLOC, e=2,
                                    p=SCOL)
            # scan tiles, lane-packed: row = n + 64*(h//32); re and im
            SA = big.tile([2 * N, BLOC * C2 * HH2], BF16, tag="SA")
            SAv = SA[:].rearrange("n (b g r h) -> n b g r h", b=BLOC, g=NG, r=G1)
            SB = big.tile([2 * N, BLOC * C2 * HH2], BF16, tag="SB")
            SBv = SB[:].rearrange("n (b g r h) -> n b g r h", b=BLOC, g=NG, r=G1)

            for b in range(BLOC):   # zero pad columns (both halves)
                for e in range(2):
                    nc.vector.memset(Sstv[:, b, e, 0, :], 0.0)

            # ---------------- parameters ----------------
            # x-path params on the SP queue (needed first); big D-phase
            # params issued later on the Pool/SWDGE queue (bypasses HWDGE).
            wb_sb = cpar.tile([H + 1, H], BF16, tag="wb")
            nc.sync.dma_start(wb_sb[:], WB.ap())

            # FiLM prologue -> per-(h,b) scalar columns
            fwt = cpar.tile([CD, 2 * H], FP32, tag="fwt")
            nc.gpsimd.dma_start(fwt[:], film_WT.ap())
            fbl = cpar.tile([1, 2 * H], FP32, tag="fbl")
            nc.gpsimd.dma_start(fbl[:], film_bl.ap())
            ct = cpar.tile([CD, BLOC], FP32, tag="ct")
            nc.gpsimd.dma_start(ct[:], condT.ap())
            on1 = cpar.tile([1, BLOC], FP32, tag="on1")
            nc.gpsimd.dma_start(on1[:], ones1.ap())
            gps_t = ps_d.tile([128, 2 * T], FP32, tag="pd")
            gps = gps_t[0:2 * H, 0:BLOC]
            nc.tensor.matmul(gps, fwt[:], ct[:], start=True, stop=False)
            nc.tensor.matmul(gps, fbl[:], on1[:], start=False, stop=True)
            gb_sb = cpar.tile([2 * H, BLOC], FP32, tag="gb")
            nc.scalar.copy(gb_sb[:], gps)
            nc.gpsimd.dma_start(
                gb_scratch.ap().rearrange("(r b) -> r b", b=BLOC), gb_sb[:])
            # paired broadcast columns: col (q, hp, b) rows 0:64 = gb[q,2hp,b],
            # rows 64:128 = gb[q,2hp+1,b]
            gbv = gb_scratch.ap().rearrange(
                "(q hp e b) -> q hp e b", q=2, hp=H // 2, e=2)
            gbP = cpar.tile([128, 2 * (H // 2) * BLOC], FP32, tag="gbP")
            gbPv = gbP[:].rearrange("p (q hp b) -> p q hp b", q=2, hp=H // 2)
            nc.gpsimd.dma_start(
                gbPv[0:N], gbv[:, :, 0, :][None].broadcast_to(
                    [N, 2, H // 2, BLOC]))
            nc.gpsimd.dma_start(
                gbPv[N:128], gbv[:, :, 1, :][None].broadcast_to(
                    [N, 2, H // 2, BLOC]))

            # ---------------- phase A: u = gelu(W x + b) ----------------
            def phase_a(b):
                XC = 8      # chunks per x DMA; 4 chunks per psum tile
                for cg in range(C // XC):
                    xt = xa.tile([H + 1, XC * T], BF16, tag="xt")
                    nc.sync.dma_start(
                        xt[:], xbf.ap()[b, :, cg * XC * T:(cg + 1) * XC * T])
                    for half in range(2):
                        pa = ps_a.tile([T, 4 * H], FP32)
                        for cc in range(4):
                            cx = half * 4 + cc
                            nc.tensor.matmul(
                                pa[:, cc * H:(cc + 1) * H],
                                xt[:, cx * T:(cx + 1) * T], wb_sb[:],
                                start=True, stop=True, skip_group_check=True)
                        c0 = cg * XC + half * 4
                        dst = uv[:, :, b, c0:c0 + 4].rearrange("t h c -> t c h")
                        nc.scalar.activation(dst, pa[:], AF.Gelu)

            phase_a(0)

            # B-phase params (needed right after A(b=0)): Pool queue
            va_sb = {}
            for nm, tens in (("lr", VA_lo_re), ("hr", VA_hi_re),
                             ("li", VA_lo_im), ("hi", VA_hi_im)):
                t_ = cpar.tile([T, H * N], BF16, tag="va" + nm)
                nc.gpsimd.dma_start(t_[:].rearrange("t (h f) -> t h f", h=H),
                                    tens.ap())
                va_sb[nm] = t_
            mu2 = cpar.tile([2 * N, 2 * HH2], BF16, tag="mu2")
            nc.gpsimd.dma_start(mu2[:, 0:HH2], MU2_re.ap())
            nc.gpsimd.dma_start(mu2[:, HH2:], MU2_im.ap())
            nu2 = cpar.tile([2 * N, 2 * HH2], BF16, tag="nu2")
            nc.gpsimd.dma_start(nu2[:, 0:HH2], NU2_re.ap())
            nc.gpsimd.dma_start(nu2[:, HH2:], NU2_im.ap())
            cb = cpar.tile([2 * N, 2 * (G1 - 1) * HH2], BF16, tag="cb")
            cbv = cb[:].rearrange("n (q r h) -> n q r h", q=2, r=G1 - 1)
            nc.gpsimd.dma_start(cbv[:, 0], CB_re.ap())
            nc.gpsimd.dma_start(cbv[:, 1], CB_im.ap())

            phase_a(1)

            # D-phase params (needed ~after the first scan): Pool queue
            kt_sb = cpar.tile([T, H * T], BF16, tag="kt")
            nc.gpsimd.dma_start(kt_sb[:].rearrange("t (h f) -> t h f", h=H),
                                KT.ap())
            kt2_sb = cpar.tile([T, H * T], BF16, tag="kt2")
            nc.gpsimd.dma_start(kt2_sb[:].rearrange("t (h f) -> t h f", h=H),
                                KT2.ap())
            pst_sb = cpar.tile([2 * N, H * T], BF16, tag="pst")
            nc.gpsimd.dma_start(pst_sb[:].rearrange("n (h f) -> n h f", h=H),
                                PST.ap())
            pmt_sb = cpar.tile([2 * N, H * T], BF16, tag="pmt")
            nc.gpsimd.dma_start(pmt_sb[:].rearrange("n (h f) -> n h f", h=H),
                                PMT.ap())

            # ---------------- per-b: phase B + scan + phase D ----------------
            mre = mu2[:, 0:HH2]; mim = mu2[:, HH2:]
            nre = nu2[:, 0:HH2]; nim = nu2[:, HH2:]

            def cfma(dre, dim_, wre, wim, sre, sim_, fshape):
                """d += w * s (complex); w broadcast tiles, all bf16."""
                nf = int(np.prod(fshape))
                t1 = tmp.tile([2 * N, NG * HH2], BF16, tag="t1")
                t2 = tmp.tile([2 * N, NG * HH2], BF16, tag="t2")
                a = t1[:, :nf].rearrange("n (x y) -> n x y", x=fshape[0])
                bq = t2[:, :nf].rearrange("n (x y) -> n x y", x=fshape[0])
                nc.vector.tensor_mul(a, wre, sre)
                nc.vector.tensor_mul(bq, wim, sim_)
                nc.vector.tensor_sub(a, a, bq)
                nc.vector.tensor_add(dre, dre, a)
                nc.vector.tensor_mul(a, wre, sim_)
                nc.vector.tensor_mul(bq, wim, sre)
                nc.vector.tensor_add(a, a, bq)
                nc.vector.tensor_add(dim_, dim_, a)

            for b in range(BLOC):
                # scan views [128, g, r, h32]
                sreV = SAv[:, b]
                simV = SBv[:, b]

                # ---- phase B: V pair-summaries (lane-packed by h-half) ----
                for hg in range(HH2 // HGB):
                    h0 = hg * HGB
                    pv = ps_b.tile([2 * N, 2 * HGB * C2], FP32)   # (q, i, p)
                    for hh in range(HGB):
                        for e in range(2):
                            h = e * HH2 + h0 + hh
                            rows = slice(e * N, e * N + N)
                            rhs_e = uq[:, h, b, :, 0]
                            rhs_o = uq[:, h, b, :, 1]
                            for q, (lo, hi) in enumerate(
                                    (("lr", "hr"), ("li", "hi"))):
                                sl = slice((q * HGB + hh) * C2,
                                           (q * HGB + hh + 1) * C2)
                                vlo = va_sb[lo][:, h * N:(h + 1) * N]
                                vhi = va_sb[hi][:, h * N:(h + 1) * N]
                                nc.tensor.matmul(pv[rows, sl], vlo, rhs_e,
                                                 start=True, stop=False,
                                                 skip_group_check=True)
                                nc.tensor.matmul(pv[rows, sl], vhi, rhs_o,
                                                 start=False, stop=True,
                                                 skip_group_check=True)
                    pvv = pv[:].rearrange("n (q h p) -> n q h p", q=2, h=HGB)
                    dvr = SAv[:, b, :, :, h0:h0 + HGB].rearrange(
                        "n g r h -> n h (g r)")
                    nc.scalar.activation(dvr, pvv[:, 0], AF.Copy)
                    dvi = SBv[:, b, :, :, h0:h0 + HGB].rearrange(
                        "n g r h -> n h (g r)")
                    nc.scalar.activation(dvi, pvv[:, 1], AF.Copy)

                # ---- hierarchical scan over 64 pairs ----
                for r in range(1, G1):
                    cfma(sreV[:, :, r, :], simV[:, :, r, :],
                         mre[:, None, :].broadcast_to([2 * N, NG, HH2]),
                         mim[:, None, :].broadcast_to([2 * N, NG, HH2]),
                         sreV[:, :, r - 1, :], simV[:, :, r - 1, :],
                         (NG, HH2))
                for g in range(1, NG):
                    cfma(sreV[:, g, G1 - 1, :][:, None, :],
                         simV[:, g, G1 - 1, :][:, None, :],
                         nre[:, None, :].broadcast_to([2 * N, 1, HH2]),
                         nim[:, None, :].broadcast_to([2 * N, 1, HH2]),
                         sreV[:, g - 1, G1 - 1, :][:, None, :],
                         simV[:, g - 1, G1 - 1, :][:, None, :],
                         (1, HH2))
                for r in range(G1 - 1):
                    cfma(sreV[:, 1:NG, r, :], simV[:, 1:NG, r, :],
                         cbv[:, 0, r][:, None, :].broadcast_to(
                             [2 * N, NG - 1, HH2]),
                         cbv[:, 1, r][:, None, :].broadcast_to(
                             [2 * N, NG - 1, HH2]),
                         sreV[:, 0:NG - 1, G1 - 1, :],
                         simV[:, 0:NG - 1, G1 - 1, :],
                         (NG - 1, HH2))

                # ---- gather into stacked carry layout (SBUF->SBUF DMAs) ----
                sa = SAv[:, b].rearrange("n g r h -> n (g r h)")
                sb_ = SBv[:, b].rearrange("n g r h -> n (g r h)")
                for e in range(2):
                    nc.sync.dma_start(
                        Sstv[0:N, b, e, 1:SCOL, :].rearrange("n p h -> n (p h)"),
                        sa[e * N:(e + 1) * N, :])
                    nc.sync.dma_start(
                        Sstv[N:2 * N, b, e, 1:SCOL, :].rearrange(
                            "n p h -> n (p h)"),
                        sb_[e * N:(e + 1) * N, :])

                # ---- phase D: conv + carry + FiLM + residual + store ----
                # h-pairs share one [128, 256] psum tile: even h rows 0:64,
                # odd h rows 64:128; one act per pair (gbP columns)
                HP = HG // 2     # 4 pairs per store group
                for hg in range(H // HG):
                    hp0 = hg * HP
                    yt = ytp.tile([128, HP * 2 * T], BF16, tag="yt")
                    rx = rxp.tile([128, HP * 2 * T], BF16, tag="rx")
                    nc.sync.dma_start(
                        rx[:].rearrange("p (h f) -> p h f", h=HP),
                        xrt.ap()[b, :, hp0:hp0 + HP, :])
                    for hh in range(HP):
                        hp = hp0 + hh
                        pdt = ps_d.tile([128, 2 * T], FP32, tag="pd")
                        for e in range(2):
                            h = 2 * hp + e
                            pd = pdt[64 * e:64 * (e + 1), :]
                            lhs_e = uq[:, h, b, :, 0]
                            lhs_o = uq[:, h, b, :, 1]
                            kth = kt_sb[:, h * T:(h + 1) * T]
                            kt2h = kt2_sb[:, h * T:(h + 1) * T]
                            psth = pst_sb[:, h * T:(h + 1) * T]
                            pmth = pmt_sb[:, h * T:(h + 1) * T]
                            ssth = Sstv[:, b, h // HH2, 0:C2, h % HH2]
                            nc.tensor.matmul(pd[:, 0:T], lhs_e, kth,
                                             start=True, stop=False,
                                             skip_group_check=True)
                            nc.tensor.matmul(pd[:, 0:T], ssth, psth,
                                             start=False, stop=True,
                                             skip_group_check=True)
                            nc.tensor.matmul(pd[:, T:2 * T], lhs_o, kth,
                                             start=True, stop=False,
                                             skip_group_check=True)
                            nc.tensor.matmul(pd[:, T:2 * T], lhs_e, kt2h,
                                             start=False, stop=False,
                                             skip_group_check=True)
                            nc.tensor.matmul(pd[:, T:2 * T], ssth, pmth,
                                             start=False, stop=True,
                                             skip_group_check=True)
                        sc = (0 * (H // 2) + hp) * BLOC + b
                        bc = (1 * (H // 2) + hp) * BLOC + b
                        nc.scalar.activation(
                            yt[:, hh * 2 * T:(hh + 1) * 2 * T], pdt[:],
                            AF.Gelu,
                            bias=gbP[:, bc:bc + 1], scale=gbP[:, sc:sc + 1])
                    eng = nc.gpsimd if b == 0 else nc.vector
                    eng.tensor_add(yt[:], yt[:], rx[:])
                    nc.sync.dma_start(
                        y_out.ap()[b].rearrange(
                            "(hp e) (p f) -> e p hp f", e=2, f=2 * T)
                        [:, :, hp0:hp0 + HP, :],
                        yt[:].rearrange("p (h f) -> p h f", h=HP))

    _split_tail_drain_waits(nc)
    return nc


def kernel(**inputs):
    import ml_dtypes
    if "k" not in _CACHE:
        _CACHE["k"] = _build()
    nc = _CACHE["k"]

    hp = _host_params(
        inputs["log_dt"], inputs["log_A_real"], inputs["A_imag"],
        inputs["C_re"], inputs["C_im"], inputs["D"],
        inputs["W_lin"], inputs["b_lin"])

    x = np.asarray(inputs["x"], dtype=np.float32)
    res_w = np.asarray(inputs["res_w"], dtype=np.float32)
    cond = np.ascontiguousarray(
        np.asarray(inputs["conditional_information"], dtype=np.float32))
    film_W = np.asarray(inputs["film_W"], dtype=np.float32)
    film_b = np.asarray(inputs["film_b"], dtype=np.float32)

    bf16 = ml_dtypes.bfloat16
    xb = x.astype(bf16)                                       # [B,H,L]
    ones_row = np.ones((B, 1, L), dtype=bf16)
    xbf = np.ascontiguousarray(np.concatenate([xb, ones_row], axis=1))
    rx = (x * res_w[None, :, None]).astype(bf16)
    # [b, hp, e, c2, f] -> [b, (e c2), hp, f]
    xrt = np.ascontiguousarray(
        rx.reshape(B, H // 2, 2, C2, 2 * T).transpose(0, 2, 3, 1, 4)
        .reshape(B, 2 * C2, H // 2, 2 * T))

    common = dict(hp)
    common["film_WT"] = np.ascontiguousarray(film_W.T)
    common["film_bl"] = np.ascontiguousarray(film_b[None, :])
    common["ones1"] = np.ones((1, BLOC), np.float32)

    in_maps = []
    for c_ in range(NCORES):
        m = dict(common)
        m["xbf"] = np.ascontiguousarray(xbf[c_ * BLOC:(c_ + 1) * BLOC])
        m["xrt"] = np.ascontiguousarray(xrt[c_ * BLOC:(c_ + 1) * BLOC])
        m["condT"] = np.ascontiguousarray(cond[c_ * BLOC:(c_ + 1) * BLOC].T)
        in_maps.append(m)

    res = run_bass_kernel_spmd(nc, in_maps, core_ids=list(range(NCORES)))
    out = np.concatenate(
        [np.asarray(res.results[c_]["y_out"]) for c_ in range(NCORES)], axis=0)
    return out.astype(np.float32)


# revision 35
# speedup vs baseline: 3.4067x; 1.0600x over previous
"""Trainium2 Bass kernel for nn_Block_19301583028789 (v2).

Pipeline per batch: channel Linear -> erf-GELU -> S4D (chunked linear
recurrence, exact) -> FiLM -> erf-GELU -> per-channel residual.

v2 redesign vs v1 (587us):
- all matmuls bf16 (1 cyc/row), x pre-cast to bf16 host-side
- W=2 pair-level state scan: 64 scanned states instead of 128 (pair
  summaries built by accumulating matmuls at no extra PE cost)
- [c,t]-form conv output (out[c,t] = u^T KT + S^T P), chunk parity in the
  free dim -> no PE transposes, no extra PSUM->SBUF copy passes
- re/im carry contraction stacked on 128 partitions (1 matmul, not 2);
  im half moved across lanes by one SBUF->SBUF DMA after the scan
- few, large DMAs (HWDGE fixed cost is ~625ns per DMA instruction)
- residual pre-scaled by res_w host-side; bf16 output, host upcasts

Sharding: data-parallel over batch B=16 across 8 cores (2 per core).
"""

import numpy as np

import concourse.bass as bass
import concourse.tile as tile
import concourse.mybir as mybir
from concourse.bass_utils import run_bass_kernel_spmd

B, H, L = 16, 64, 16384
N, CD = 64, 32
T = 128
C = L // T            # 128 chunks
C2 = C // 2           # 64 chunk pairs (scan granularity)
G1 = 8                # pairs per scan group
NG = C2 // G1         # 8 groups
NCORES = 8
BLOC = B // NCORES    # 2
SCOL = C2 + 1         # pad column + 64 pair states
FP32 = mybir.dt.float32
BF16 = mybir.dt.bfloat16
AF = mybir.ActivationFunctionType

_CACHE = {}


def _split_tail_drain_waits(nc, max_waits=1):
    """Walrus TPB_CTRL lowering only accepts 1 sync-wait per Drain/NoOp."""
    for fn in nc.m.functions:
        for blk in fn.blocks:
            insts = blk.instructions
            i = 0
            while i < len(insts):
                inst = insts[i]
                si = inst.sync_info
                if si is not None and len(si.on_wait) > max_waits:
                    extra = list(si.on_wait[:-max_waits])
                    keep = list(si.on_wait[-max_waits:])
                    nops = [
                        mybir.InstNoOp(
                            name=f"{inst.name}-waitsplit{k}",
                            opcode="NoOp",
                            engine=inst.engine,
                            sync_info=mybir.SyncInfo(on_wait=[w], on_update=[]),
                        )
                        for k, w in enumerate(extra)
                    ]
                    si.on_wait = keep
                    for k, nop in enumerate(nops):
                        insts.insert(i + k, nop)
                    i += len(nops)
                i += 1


def _host_params(log_dt, log_A_real, A_imag, C_re, C_im, D, W_lin, b_lin):
    """Parameter-derived constant matrices (fp64 host math), bf16-packed."""
    import ml_dtypes
    bf = lambda a: np.ascontiguousarray(
        np.asarray(a, dtype=np.float64).astype(np.float32).astype(ml_dtypes.bfloat16))

    dt = np.exp(log_dt.astype(np.float64))[:, None]            # [H,1]
    A = -np.exp(log_A_real.astype(np.float64)) + 1j * A_imag.astype(np.float64)
    dtA = A * dt                                               # [H,N]
    coef = (C_re.astype(np.float64) + 1j * C_im.astype(np.float64)) \
        * (np.exp(dtA) - 1.0) / A                              # [H,N]

    ks = np.arange(2 * T + 2)
    lp = np.exp(dtA[:, :, None] * ks[None, None, :])           # [H,N,2T+2]

    # local-conv kernel, D folded into tap 0
    K = 2.0 * np.real(np.einsum("hn,hnm->hm", coef, lp[:, :, :T]))  # [H,T]
    K[:, 0] += D.astype(np.float64)
    # KT[t', h, t] = K_h[t - t'] for t >= t'   (rhs of out[c,t] local conv)
    idx = np.arange(T)
    dmat = idx[None, :] - idx[:, None]                         # [t', t]
    KT = np.where(dmat[None] >= 0, K[:, np.clip(dmat, 0, T - 1)], 0.0)  # [H,t',t]
    KT = np.transpose(KT, (1, 0, 2))                           # [t',H,t]
    # KT2: kernel taps T..2T-1 = carry from the immediately-preceding chunk
    # (K2[d] = 2 Re sum_n coef lam^d, d = T + t - t', dense)
    K2 = 2.0 * np.real(np.einsum("hn,hnm->hm", coef, lp[:, :, :2 * T]))  # [H,2T]
    KT2 = K2[:, T + dmat]                                      # [H,t',t]
    KT2 = np.transpose(KT2, (1, 0, 2))

    # pair-summary lhsT tables (V[p] = sum over 2 chunks of lam^(2T-1-tau) u)
    VA_lo = np.transpose(lp[:, :, ::-1][:, :, 2:T + 2], (2, 0, 1))      # lam^(2T-1-t) [t,H,N]
    VA_hi = np.transpose(lp[:, :, ::-1][:, :, T + 2:2 * T + 2], (2, 0, 1))  # lam^(T-1-t)

    # carry rhs, stacked (re | -im): y[c,t] += Re(sum_n S[n] P[n,t])
    P = 2.0 * coef[:, :, None] * lp[:, :, 1:T + 1]             # [H,N,T]
    mu = lp[:, :, T]                                           # lam^T [H,N]
    Pmu = P * mu[:, :, None]
    PST = np.concatenate([P.real, -P.imag], axis=1)            # [H,2N,T]
    PMT = np.concatenate([Pmu.real, -Pmu.imag], axis=1)
    PST = np.transpose(PST, (1, 0, 2))                         # [2N,H,T]
    PMT = np.transpose(PMT, (1, 0, 2))

    # scan coefficients over pairs: ratio nu = mu^2 = lam^(2T)
    # lane-packed layout: row = n + 64*(h // 32), col = h % 32
    def lanepack(a_nh):                                        # [N,H] -> [128,32]
        return a_nh.reshape(N, 2, H // 2).transpose(1, 0, 2).reshape(2 * N, H // 2)

    nu = lp[:, :, 2 * T]                                       # [H,N]
    nuP = nu[None, :, :] ** (1 + np.arange(G1)[:, None, None]) # [G1,H,N]
    MU2_re = lanepack(nu.real.T); MU2_im = lanepack(nu.imag.T)
    NU2_re = lanepack(nuP[G1 - 1].real.T); NU2_im = lanepack(nuP[G1 - 1].imag.T)
    CB_re = np.stack([lanepack(nuP[r].real.T) for r in range(G1 - 1)], 1)
    CB_im = np.stack([lanepack(nuP[r].imag.T) for r in range(G1 - 1)], 1)  # [128,G1-1,32]

    WB = np.concatenate([W_lin.T.astype(np.float64),
                         b_lin.astype(np.float64)[None, :]], 0)  # [H+1,H]

    return {
        "WB": bf(WB), "KT": bf(KT), "KT2": bf(KT2),
        "VA_lo_re": bf(VA_lo.real), "VA_lo_im": bf(VA_lo.imag),
        "VA_hi_re": bf(VA_hi.real), "VA_hi_im": bf(VA_hi.imag),
        "PST": bf(PST), "PMT": bf(PMT),
        "MU2_re": bf(MU2_re), "MU2_im": bf(MU2_im),
        "NU2_re": bf(NU2_re), "NU2_im": bf(NU2_im),
        "CB_re": bf(CB_re), "CB_im": bf(CB_im),
    }


def _build():
    nc = bass.Bass("TRN2", target_bir_lowering=False, debug=False)

    def din(name, shape, dtype=BF16):
        return nc.dram_tensor(name, list(shape), dtype, kind="ExternalInput")

    xbf = din("xbf", [BLOC, H + 1, L])           # x bf16 + ones row (phase A)
    # res_w*x in the h-paired store layout: [b, (half c2), hp, 256] bf16
    xrt = din("xrt", [BLOC, 2 * C2, H // 2, 2 * T])
    WB = din("WB", [H + 1, H])
    KT = din("KT", [T, H, T])
    KT2 = din("KT2", [T, H, T])
    VA_lo_re = din("VA_lo_re", [T, H, N]); VA_lo_im = din("VA_lo_im", [T, H, N])
    VA_hi_re = din("VA_hi_re", [T, H, N]); VA_hi_im = din("VA_hi_im", [T, H, N])
    PST = din("PST", [2 * N, H, T]); PMT = din("PMT", [2 * N, H, T])
    HH2 = H // 2
    MU2_re = din("MU2_re", [2 * N, HH2]); MU2_im = din("MU2_im", [2 * N, HH2])
    NU2_re = din("NU2_re", [2 * N, HH2]); NU2_im = din("NU2_im", [2 * N, HH2])
    CB_re = din("CB_re", [2 * N, G1 - 1, HH2])
    CB_im = din("CB_im", [2 * N, G1 - 1, HH2])
    film_WT = din("film_WT", [CD, 2 * H], FP32)
    film_bl = din("film_bl", [1, 2 * H], FP32)
    condT = din("condT", [CD, BLOC], FP32)
    ones1 = din("ones1", [1, BLOC], FP32)
    # paired layout: [e, (q, hp, b)] so partition-broadcast rows are contiguous
    gb_scratch = nc.dram_tensor("gb_scratch", [2, 2 * (H // 2) * BLOC], FP32)
    y_out = nc.dram_tensor("y_out", [BLOC, H, L], BF16, kind="ExternalOutput")

    HGB = 4                      # h batch for phase B psum
    HG = 8                       # h batch for phase D store
    with tile.TileContext(nc) as tc:
        with (
            tc.tile_pool(name="big", bufs=1) as big,
            tc.tile_pool(name="cpar", bufs=1) as cpar,
            tc.tile_pool(name="xa", bufs=4) as xa,
            tc.tile_pool(name="tmp", bufs=1) as tmp,
            tc.tile_pool(name="yt", bufs=3) as ytp,
            tc.tile_pool(name="rx", bufs=3) as rxp,
            tc.tile_pool(name="ps_a", bufs=2, space="PSUM") as ps_a,
            tc.tile_pool(name="ps_b", bufs=2, space="PSUM") as ps_b,
            tc.tile_pool(name="ps_d", bufs=4, space="PSUM") as ps_d,
        ):
            # ---------------- resident tensors ----------------
            u = big.tile([T, H * BLOC * C], BF16, tag="u")      # [t,(h,b,c)]
            uv = u[:].rearrange("t (h b c) -> t h b c", h=H, b=BLOC)
            uq = u[:].rearrange("t (h b p q) -> t h b p q", h=H, b=BLOC, q=2)
            # stacked carry lhsT: rows 0:64 S_re, 64:128 S_im;
            # cols (b, hh, pad+p, h32) with h = hh*32 + h32
            Sst = big.tile([2 * N, BLOC * 2 * SCOL * HH2], BF16, tag="Sst")
            Sstv = Sst[:].rearrange("n (b e p h) -> n b e p h", b=BLOC, e=2,
                                    p=SCOL)
            # scan tiles, lane-packed: row = n + 64*(h//32); re and im
            SA = big.tile([2 * N, BLOC * C2 * HH2], BF16, tag="SA")
            SAv = SA[:].rearrange("n (b g r h) -> n b g r h", b=BLOC, g=NG, r=G1)
            SB = big.tile([2 * N, BLOC * C2 * HH2], BF16, tag="SB")
            SBv = SB[:].rearrange("n (b g r h) -> n b g r h", b=BLOC, g=NG, r=G1)

            for b in range(BLOC):   # zero pad columns (both halves)
                for e in range(2):
                    nc.vector.memset(Sstv[:, b, e, 0, :], 0.0)

            # ---------------- parameters ----------------
            # x-path params on the SP queue (needed first); big D-phase
            # params issued later on the Pool/SWDGE queue (bypasses HWDGE).
            wb_sb = cpar.tile([H + 1, H], BF16, tag="wb")
            nc.sync.dma_start(wb_sb[:], WB.ap())

            # FiLM prologue -> per-(h,b) scalar columns
            fwt = cpar.tile([CD, 2 * H], FP32, tag="fwt")
            nc.gpsimd.dma_start(fwt[:], film_WT.ap())
            fbl = cpar.tile([1, 2 * H], FP32, tag="fbl")
            nc.gpsimd.dma_start(fbl[:], film_bl.ap())
            ct = cpar.tile([CD, BLOC], FP32, tag="ct")
            nc.gpsimd.dma_start(ct[:], condT.ap())
            on1 = cpar.tile([1, BLOC], FP32, tag="on1")
            nc.gpsimd.dma_start(on1[:], ones1.ap())
            gps_t = ps_d.tile([128, 2 * T], FP32, tag="pd")
            gps = gps_t[0:2 * H, 0:BLOC]
            nc.tensor.matmul(gps, fwt[:], ct[:], start=True, stop=False)
            nc.tensor.matmul(gps, fbl[:], on1[:], start=False, stop=True)
            gb_sb = cpar.tile([2 * H, BLOC], FP32, tag="gb")
            nc.scalar.copy(gb_sb[:], gps)
            # film rows are host-permuted to (e, q, hp) order, so this store
            # lands directly in the paired [e, (q, hp, b)] layout
            nc.gpsimd.dma_start(
                gb_scratch.ap().rearrange("e (f b) -> (e f) b", b=BLOC),
                gb_sb[:])
            # paired broadcast columns: col (q, hp, b) rows 0:64 = gb[q,2hp,b],
            # rows 64:128 = gb[q,2hp+1,b]
            gbP = cpar.tile([128, 2 * (H // 2) * BLOC], FP32, tag="gbP")
            nc.gpsimd.dma_start(
                gbP[0:N, :], gb_scratch.ap()[0][None, :]
                .broadcast_to([N, 2 * (H // 2) * BLOC]))
            nc.gpsimd.dma_start(
                gbP[N:128, :], gb_scratch.ap()[1][None, :]
                .broadcast_to([N, 2 * (H // 2) * BLOC]))

            # ---------------- phase A: u = gelu(W x + b) ----------------
            def phase_a(b):
                XC = 8      # chunks per x DMA; 4 chunks per psum tile
                for cg in range(C // XC):
                    xt = xa.tile([H + 1, XC * T], BF16, tag="xt")
                    nc.sync.dma_start(
                        xt[:], xbf.ap()[b, :, cg * XC * T:(cg + 1) * XC * T])
                    for half in range(2):
                        pa = ps_a.tile([T, 4 * H], FP32)
                        for cc in range(4):
                            cx = half * 4 + cc
                            nc.tensor.matmul(
                                pa[:, cc * H:(cc + 1) * H],
                                xt[:, cx * T:(cx + 1) * T], wb_sb[:],
                                start=True, stop=True, skip_group_check=True)
                        c0 = cg * XC + half * 4
                        dst = uv[:, :, b, c0:c0 + 4].rearrange("t h c -> t c h")
                        nc.scalar.activation(dst, pa[:], AF.Gelu)

            phase_a(0)

            # B-phase params (needed right after A(b=0)): Pool queue
            va_sb = {}
            for nm, tens in (("lr", VA_lo_re), ("hr", VA_hi_re),
                             ("li", VA_lo_im), ("hi", VA_hi_im)):
                t_ = cpar.tile([T, H * N], BF16, tag="va" + nm)
                nc.gpsimd.dma_start(t_[:].rearrange("t (h f) -> t h f", h=H),
                                    tens.ap())
                va_sb[nm] = t_
            mu2 = cpar.tile([2 * N, 2 * HH2], BF16, tag="mu2")
            nc.gpsimd.dma_start(mu2[:, 0:HH2], MU2_re.ap())
            nc.gpsimd.dma_start(mu2[:, HH2:], MU2_im.ap())
            nu2 = cpar.tile([2 * N, 2 * HH2], BF16, tag="nu2")
            nc.gpsimd.dma_start(nu2[:, 0:HH2], NU2_re.ap())
            nc.gpsimd.dma_start(nu2[:, HH2:], NU2_im.ap())
            cb = cpar.tile([2 * N, 2 * (G1 - 1) * HH2], BF16, tag="cb")
            cbv = cb[:].rearrange("n (q r h) -> n q r h", q=2, r=G1 - 1)
            nc.gpsimd.dma_start(cbv[:, 0], CB_re.ap())
            nc.gpsimd.dma_start(cbv[:, 1], CB_im.ap())

            phase_a(1)

            # D-phase params: SP queue AFTER all x loads (in-order queue
            # naturally deprioritizes them below phase A's inputs)
            kt_sb = cpar.tile([T, H * T], BF16, tag="kt")
            nc.sync.dma_start(kt_sb[:].rearrange("t (h f) -> t h f", h=H),
                              KT.ap())
            kt2_sb = cpar.tile([T, H * T], BF16, tag="kt2")
            nc.sync.dma_start(kt2_sb[:].rearrange("t (h f) -> t h f", h=H),
                              KT2.ap())
            pst_sb = cpar.tile([2 * N, H * T], BF16, tag="pst")
            nc.sync.dma_start(pst_sb[:].rearrange("n (h f) -> n h f", h=H),
                              PST.ap())
            pmt_sb = cpar.tile([2 * N, H * T], BF16, tag="pmt")
            nc.sync.dma_start(pmt_sb[:].rearrange("n (h f) -> n h f", h=H),
                              PMT.ap())

            # ---------------- per-b: phase B + scan + phase D ----------------
            mre = mu2[:, 0:HH2]; mim = mu2[:, HH2:]
            nre = nu2[:, 0:HH2]; nim = nu2[:, HH2:]

            def cfma(dre, dim_, wre, wim, sre, sim_, fshape):
                """d += w * s (complex); w broadcast tiles, all bf16."""
                nf = int(np.prod(fshape))
                t1 = tmp.tile([2 * N, NG * HH2], BF16, tag="t1")
                t2 = tmp.tile([2 * N, NG * HH2], BF16, tag="t2")
                a = t1[:, :nf].rearrange("n (x y) -> n x y", x=fshape[0])
                bq = t2[:, :nf].rearrange("n (x y) -> n x y", x=fshape[0])
                nc.vector.tensor_mul(a, wre, sre)
                nc.vector.tensor_mul(bq, wim, sim_)
                nc.vector.tensor_sub(a, a, bq)
                nc.vector.tensor_add(dre, dre, a)
                nc.vector.tensor_mul(a, wre, sim_)
                nc.vector.tensor_mul(bq, wim, sre)
                nc.vector.tensor_add(a, a, bq)
                nc.vector.tensor_add(dim_, dim_, a)

            for b in range(BLOC):
                # scan views [128, g, r, h32]
                sreV = SAv[:, b]
                simV = SBv[:, b]

                # ---- phase B: V pair-summaries (lane-packed by h-half) ----
                for hg in range(HH2 // HGB):
                    h0 = hg * HGB
                    pv = ps_b.tile([2 * N, 2 * HGB * C2], FP32)   # (q, i, p)
                    for hh in range(HGB):
                        for e in range(2):
                            h = e * HH2 + h0 + hh
                            rows = slice(e * N, e * N + N)
                            rhs_e = uq[:, h, b, :, 0]
                            rhs_o = uq[:, h, b, :, 1]
                            for q, (lo, hi) in enumerate(
                                    (("lr", "hr"), ("li", "hi"))):
                                sl = slice((q * HGB + hh) * C2,
                                           (q * HGB + hh + 1) * C2)
                                vlo = va_sb[lo][:, h * N:(h + 1) * N]
                                vhi = va_sb[hi][:, h * N:(h + 1) * N]
                                nc.tensor.matmul(pv[rows, sl], vlo, rhs_e,
                                                 start=True, stop=False,
                                                 skip_group_check=True)
                                nc.tensor.matmul(pv[rows, sl], vhi, rhs_o,
                                                 start=False, stop=True,
                                                 skip_group_check=True)
                    pvv = pv[:].rearrange("n (q h p) -> n q h p", q=2, h=HGB)
                    dvr = SAv[:, b, :, :, h0:h0 + HGB].rearrange(
                        "n g r h -> n h (g r)")
                    nc.scalar.activation(dvr, pvv[:, 0], AF.Copy)
                    dvi = SBv[:, b, :, :, h0:h0 + HGB].rearrange(
                        "n g r h -> n h (g r)")
                    nc.scalar.activation(dvi, pvv[:, 1], AF.Copy)

                # ---- hierarchical scan over 64 pairs ----
                for r in range(1, G1):
                    cfma(sreV[:, :, r, :], simV[:, :, r, :],
                         mre[:, None, :].broadcast_to([2 * N, NG, HH2]),
                         mim[:, None, :].broadcast_to([2 * N, NG, HH2]),
                         sreV[:, :, r - 1, :], simV[:, :, r - 1, :],
                         (NG, HH2))
                for g in range(1, NG):
                    cfma(sreV[:, g, G1 - 1, :][:, None, :],
                         simV[:, g, G1 - 1, :][:, None, :],
                         nre[:, None, :].broadcast_to([2 * N, 1, HH2]),
                         nim[:, None, :].broadcast_to([2 * N, 1, HH2]),
                         sreV[:, g - 1, G1 - 1, :][:, None, :],
                         simV[:, g - 1, G1 - 1, :][:, None, :],
                         (1, HH2))
                for r in range(G1 - 1):
                    cfma(sreV[:, 1:NG, r, :], simV[:, 1:NG, r, :],
                         cbv[:, 0, r][:, None, :].broadcast_to(
                             [2 * N, NG - 1, HH2]),
                         cbv[:, 1, r][:, None, :].broadcast_to(
                             [2 * N, NG - 1, HH2]),
                         sreV[:, 0:NG - 1, G1 - 1, :],
                         simV[:, 0:NG - 1, G1 - 1, :],
                         (NG - 1, HH2))

                # ---- gather into stacked carry layout (SBUF->SBUF DMAs) ----
                sa = SAv[:, b].rearrange("n g r h -> n (g r h)")
                sb_ = SBv[:, b].rearrange("n g r h -> n (g r h)")
                for e in range(2):
                    nc.sync.dma_start(
                        Sstv[0:N, b, e, 1:SCOL, :].rearrange("n p h -> n (p h)"),
                        sa[e * N:(e + 1) * N, :])
                    nc.sync.dma_start(
                        Sstv[N:2 * N, b, e, 1:SCOL, :].rearrange(
                            "n p h -> n (p h)"),
                        sb_[e * N:(e + 1) * N, :])

                # ---- phase D: conv + carry + FiLM + residual + store ----
                # h-pairs share one [128, 256] psum tile: even h rows 0:64,
                # odd h rows 64:128; one act per pair (gbP columns)
                HP = HG // 2     # 4 pairs per store group
                for hg in range(H // HG):
                    hp0 = hg * HP
                    yt = ytp.tile([128, HP * 2 * T], BF16, tag="yt")
                    rx = rxp.tile([128, HP * 2 * T], BF16, tag="rx")
                    nc.sync.dma_start(
                        rx[:].rearrange("p (h f) -> p h f", h=HP),
                        xrt.ap()[b, :, hp0:hp0 + HP, :])
                    for hh in range(HP):
                        hp = hp0 + hh
                        pdt = ps_d.tile([128, 2 * T], FP32, tag="pd")
                        for e in range(2):
                            h = 2 * hp + e
                            pd = pdt[64 * e:64 * (e + 1), :]
                            lhs_e = uq[:, h, b, :, 0]
                            lhs_o = uq[:, h, b, :, 1]
                            kth = kt_sb[:, h * T:(h + 1) * T]
                            kt2h = kt2_sb[:, h * T:(h + 1) * T]
                            psth = pst_sb[:, h * T:(h + 1) * T]
                            pmth = pmt_sb[:, h * T:(h + 1) * T]
                            ssth = Sstv[:, b, h // HH2, 0:C2, h % HH2]
                            nc.tensor.matmul(pd[:, 0:T], lhs_e, kth,
                                             start=True, stop=False,
                                             skip_group_check=True)
                            nc.tensor.matmul(pd[:, 0:T], ssth, psth,
                                             start=False, stop=True,
                                             skip_group_check=True)
                            nc.tensor.matmul(pd[:, T:2 * T], lhs_o, kth,
                                             start=True, stop=False,
                                             skip_group_check=True)
                            nc.tensor.matmul(pd[:, T:2 * T], lhs_e, kt2h,
                                             start=False, stop=False,
                                             skip_group_check=True)
                            nc.tensor.matmul(pd[:, T:2 * T], ssth, pmth,
                                             start=False, stop=True,
                                             skip_group_check=True)
                        sc = (0 * (H // 2) + hp) * BLOC + b
                        bc = (1 * (H // 2) + hp) * BLOC + b
                        nc.scalar.activation(
                            yt[:, hh * 2 * T:(hh + 1) * 2 * T], pdt[:],
                            AF.Gelu,
                            bias=gbP[:, bc:bc + 1], scale=gbP[:, sc:sc + 1])
                    eng = nc.gpsimd if b == 0 else nc.vector
                    eng.tensor_add(yt[:], yt[:], rx[:])
                    nc.sync.dma_start(
                        y_out.ap()[b].rearrange(
                            "(hp e) (p f) -> e p hp f", e=2, f=2 * T)
                        [:, :, hp0:hp0 + HP, :],
                        yt[:].rearrange("p (h f) -> p h f", h=HP))

    _split_tail_drain_waits(nc)
    return nc


def kernel(**inputs):
    import ml_dtypes
    if "k" not in _CACHE:
        _CACHE["k"] = _build()
    nc = _CACHE["k"]

    hp = _host_params(
        inputs["log_dt"], inputs["log_A_real"], inputs["A_imag"],
        inputs["C_re"], inputs["C_im"], inputs["D"],
        inputs["W_lin"], inputs["b_lin"])

    x = np.asarray(inputs["x"], dtype=np.float32)
    res_w = np.asarray(inputs["res_w"], dtype=np.float32)
    cond = np.ascontiguousarray(
        np.asarray(inputs["conditional_information"], dtype=np.float32))
    film_W = np.asarray(inputs["film_W"], dtype=np.float32)
    film_b = np.asarray(inputs["film_b"], dtype=np.float32)

    bf16 = ml_dtypes.bfloat16
    xb = x.astype(bf16)                                       # [B,H,L]
    ones_row = np.ones((B, 1, L), dtype=bf16)
    xbf = np.ascontiguousarray(np.concatenate([xb, ones_row], axis=1))
    rx = (x * res_w[None, :, None]).astype(bf16)
    # [b, hp, e, c2, f] -> [b, (e c2), hp, f]
    xrt = np.ascontiguousarray(
        rx.reshape(B, H // 2, 2, C2, 2 * T).transpose(0, 2, 3, 1, 4)
        .reshape(B, 2 * C2, H // 2, 2 * T))

    # permute FiLM output rows to (e, q, hp): new[e*H + q*H/2 + hp] = old[q*H + 2hp + e]
    perm = np.array([q * H + 2 * hp_ + e
                     for e in range(2) for q in range(2)
                     for hp_ in range(H // 2)])
    common = dict(hp)
    common["film_WT"] = np.ascontiguousarray(film_W.T[:, perm])
    common["film_bl"] = np.ascontiguousarray(film_b[None, perm])
    common["ones1"] = np.ones((1, BLOC), np.float32)

    in_maps = []
    for c_ in range(NCORES):
        m = dict(common)
        m["xbf"] = np.ascontiguousarray(xbf[c_ * BLOC:(c_ + 1) * BLOC])
        m["xrt"] = np.ascontiguousarray(xrt[c_ * BLOC:(c_ + 1) * BLOC])
        m["condT"] = np.ascontiguousarray(cond[c_ * BLOC:(c_ + 1) * BLOC].T)
        in_maps.append(m)

    res = run_bass_kernel_spmd(nc, in_maps, core_ids=list(range(NCORES)))
    out = np.concatenate(
        [np.asarray(res.results[c_]["y_out"]) for c_ in range(NCORES)], axis=0)
    return out.astype(np.float32)


# revision 44
# speedup vs baseline: 3.5619x; 1.0456x over previous
"""Trainium2 Bass kernel for nn_Block_19301583028789 (v2).

Pipeline per batch: channel Linear -> erf-GELU -> S4D (chunked linear
recurrence, exact) -> FiLM -> erf-GELU -> per-channel residual.

v2 redesign vs v1 (587us):
- all matmuls bf16 (1 cyc/row), x pre-cast to bf16 host-side
- W=2 pair-level state scan: 64 scanned states instead of 128 (pair
  summaries built by accumulating matmuls at no extra PE cost)
- [c,t]-form conv output (out[c,t] = u^T KT + S^T P), chunk parity in the
  free dim -> no PE transposes, no extra PSUM->SBUF copy passes
- re/im carry contraction stacked on 128 partitions (1 matmul, not 2);
  im half moved across lanes by one SBUF->SBUF DMA after the scan
- few, large DMAs (HWDGE fixed cost is ~625ns per DMA instruction)
- residual pre-scaled by res_w host-side; bf16 output, host upcasts

Sharding: data-parallel over batch B=16 across 8 cores (2 per core).
"""

import numpy as np

import concourse.bass as bass
import concourse.tile as tile
import concourse.mybir as mybir
from concourse.bass_utils import run_bass_kernel_spmd

B, H, L = 16, 64, 16384
N, CD = 64, 32
T = 128
C = L // T            # 128 chunks
C2 = C // 2           # 64 chunk pairs (scan granularity)
G1 = 8                # pairs per scan group
NG = C2 // G1         # 8 groups
NCORES = 8
BLOC = B // NCORES    # 2
SCOL = C2 + 1         # pad column + 64 pair states
FP32 = mybir.dt.float32
BF16 = mybir.dt.bfloat16
AF = mybir.ActivationFunctionType

_CACHE = {}


def _split_tail_drain_waits(nc, max_waits=1):
    """Walrus TPB_CTRL lowering only accepts 1 sync-wait per Drain/NoOp."""
    for fn in nc.m.functions:
        for blk in fn.blocks:
            insts = blk.instructions
            i = 0
            while i < len(insts):
                inst = insts[i]
                si = inst.sync_info
                if si is not None and len(si.on_wait) > max_waits:
                    extra = list(si.on_wait[:-max_waits])
                    keep = list(si.on_wait[-max_waits:])
                    nops = [
                        mybir.InstNoOp(
                            name=f"{inst.name}-waitsplit{k}",
                            opcode="NoOp",
                            engine=inst.engine,
                            sync_info=mybir.SyncInfo(on_wait=[w], on_update=[]),
                        )
                        for k, w in enumerate(extra)
                    ]
                    si.on_wait = keep
                    for k, nop in enumerate(nops):
                        insts.insert(i + k, nop)
                    i += len(nops)
                i += 1


def _host_params(log_dt, log_A_real, A_imag, C_re, C_im, D, W_lin, b_lin):
    """Parameter-derived constant matrices (fp64 host math), bf16-packed."""
    import ml_dtypes
    bf = lambda a: np.ascontiguousarray(
        np.asarray(a, dtype=np.float64).astype(np.float32).astype(ml_dtypes.bfloat16))

    dt = np.exp(log_dt.astype(np.float64))[:, None]            # [H,1]
    A = -np.exp(log_A_real.astype(np.float64)) + 1j * A_imag.astype(np.float64)
    dtA = A * dt                                               # [H,N]
    coef = (C_re.astype(np.float64) + 1j * C_im.astype(np.float64)) \
        * (np.exp(dtA) - 1.0) / A                              # [H,N]

    ks = np.arange(2 * T + 2)
    lp = np.exp(dtA[:, :, None] * ks[None, None, :])           # [H,N,2T+2]

    # local-conv kernel, D folded into tap 0
    K = 2.0 * np.real(np.einsum("hn,hnm->hm", coef, lp[:, :, :T]))  # [H,T]
    K[:, 0] += D.astype(np.float64)
    # KT[t', h, t] = K_h[t - t'] for t >= t'   (rhs of out[c,t] local conv)
    idx = np.arange(T)
    dmat = idx[None, :] - idx[:, None]                         # [t', t]
    KT = np.where(dmat[None] >= 0, K[:, np.clip(dmat, 0, T - 1)], 0.0)  # [H,t',t]
    KT = np.transpose(KT, (1, 0, 2))                           # [t',H,t]
    # KT2: kernel taps T..2T-1 = carry from the immediately-preceding chunk
    # (K2[d] = 2 Re sum_n coef lam^d, d = T + t - t', dense)
    K2 = 2.0 * np.real(np.einsum("hn,hnm->hm", coef, lp[:, :, :2 * T]))  # [H,2T]
    KT2 = K2[:, T + dmat]                                      # [H,t',t]
    KT2 = np.transpose(KT2, (1, 0, 2))

    # pair-summary lhsT tables (V[p] = sum over 2 chunks of lam^(2T-1-tau) u)
    VA_lo = np.transpose(lp[:, :, ::-1][:, :, 2:T + 2], (2, 0, 1))      # lam^(2T-1-t) [t,H,N]
    VA_hi = np.transpose(lp[:, :, ::-1][:, :, T + 2:2 * T + 2], (2, 0, 1))  # lam^(T-1-t)

    # carry rhs, stacked (re | -im): y[c,t] += Re(sum_n S[n] P[n,t])
    P = 2.0 * coef[:, :, None] * lp[:, :, 1:T + 1]             # [H,N,T]
    mu = lp[:, :, T]                                           # lam^T [H,N]
    Pmu = P * mu[:, :, None]
    PST = np.concatenate([P.real, -P.imag], axis=1)            # [H,2N,T]
    PMT = np.concatenate([Pmu.real, -Pmu.imag], axis=1)
    PST = np.transpose(PST, (1, 0, 2))                         # [2N,H,T]
    PMT = np.transpose(PMT, (1, 0, 2))

    # scan coefficients over pairs: ratio nu = mu^2 = lam^(2T)
    # lane-packed layout: row = n + 64*(h // 32), col = h % 32
    def lanepack(a_nh):                                        # [N,H] -> [128,32]
        return a_nh.reshape(N, 2, H // 2).transpose(1, 0, 2).reshape(2 * N, H // 2)

    nu = lp[:, :, 2 * T]                                       # [H,N]
    nuP = nu[None, :, :] ** (1 + np.arange(G1)[:, None, None]) # [G1,H,N]
    MU2_re = lanepack(nu.real.T); MU2_im = lanepack(nu.imag.T)
    NU2_re = lanepack(nuP[G1 - 1].real.T); NU2_im = lanepack(nuP[G1 - 1].imag.T)
    CB_re = np.stack([lanepack(nuP[r].real.T) for r in range(G1 - 1)], 1)
    CB_im = np.stack([lanepack(nuP[r].imag.T) for r in range(G1 - 1)], 1)  # [128,G1-1,32]

    WB = np.concatenate([W_lin.T.astype(np.float64),
                         b_lin.astype(np.float64)[None, :]], 0)  # [H+1,H]

    return {
        "WB": bf(WB), "KT": bf(KT), "KT2": bf(KT2),
        "VA_lo_re": bf(VA_lo.real), "VA_lo_im": bf(VA_lo.imag),
        "VA_hi_re": bf(VA_hi.real), "VA_hi_im": bf(VA_hi.imag),
        "PST": bf(PST), "PMT": bf(PMT),
        "MU2_re": bf(MU2_re), "MU2_im": bf(MU2_im),
        "NU2_re": bf(NU2_re), "NU2_im": bf(NU2_im),
        "CB_re": bf(CB_re), "CB_im": bf(CB_im),
    }


def _build():
    nc = bass.Bass("TRN2", target_bir_lowering=False, debug=False)

    def din(name, shape, dtype=BF16):
        return nc.dram_tensor(name, list(shape), dtype, kind="ExternalInput")

    xbf = din("xbf", [BLOC, H + 1, L])           # x bf16 + ones row (phase A)
    # res_w*x in the h-paired store layout: [b, (half c2), hp, 256] bf16
    xrt = din("xrt", [BLOC, 2 * C2, H // 2, 2 * T])
    WB = din("WB", [H + 1, H])
    KT = din("KT", [T, H, T])
    KT2 = din("KT2", [T, H, T])
    VA_lo_re = din("VA_lo_re", [T, H, N]); VA_lo_im = din("VA_lo_im", [T, H, N])
    VA_hi_re = din("VA_hi_re", [T, H, N]); VA_hi_im = din("VA_hi_im", [T, H, N])
    PST = din("PST", [2 * N, H, T]); PMT = din("PMT", [2 * N, H, T])
    HH2 = H // 2
    MU2_re = din("MU2_re", [2 * N, HH2]); MU2_im = din("MU2_im", [2 * N, HH2])
    NU2_re = din("NU2_re", [2 * N, HH2]); NU2_im = din("NU2_im", [2 * N, HH2])
    CB_re = din("CB_re", [2 * N, G1 - 1, HH2])
    CB_im = din("CB_im", [2 * N, G1 - 1, HH2])
    film_WT = din("film_WT", [CD, 2 * H], FP32)
    film_bl = din("film_bl", [1, 2 * H], FP32)
    condT = din("condT", [CD, BLOC], FP32)
    ones1 = din("ones1", [1, BLOC], FP32)
    # paired layout: [e, (q, hp, b)] so partition-broadcast rows are contiguous
    gb_scratch = nc.dram_tensor("gb_scratch", [2, 2 * (H // 2) * BLOC], FP32)
    y_out = nc.dram_tensor("y_out", [BLOC, H, L], BF16, kind="ExternalOutput")

    HGB = 4                      # h batch for phase B psum
    HG = 8                       # h batch for phase D store
    with tile.TileContext(nc) as tc:
        with (
            tc.tile_pool(name="big", bufs=1) as big,
            tc.tile_pool(name="cpar", bufs=1) as cpar,
            tc.tile_pool(name="xa", bufs=4) as xa,
            tc.tile_pool(name="tmp", bufs=1) as tmp,
            tc.tile_pool(name="yt", bufs=3) as ytp,
            tc.tile_pool(name="rx", bufs=3) as rxp,
            tc.tile_pool(name="ps_a", bufs=2, space="PSUM") as ps_a,
            tc.tile_pool(name="ps_b", bufs=2, space="PSUM") as ps_b,
            tc.tile_pool(name="ps_d", bufs=4, space="PSUM") as ps_d,
        ):
            # ---------------- resident tensors ----------------
            u = big.tile([T, H * BLOC * C], BF16, tag="u")      # [t,(h,b,c)]
            uv = u[:].rearrange("t (h b c) -> t h b c", h=H, b=BLOC)
            uq = u[:].rearrange("t (h b p q) -> t h b p q", h=H, b=BLOC, q=2)
            # stacked carry lhsT: rows 0:64 S_re, 64:128 S_im;
            # cols (b, hh, pad+p, h32) with h = hh*32 + h32
            Sst = big.tile([2 * N, BLOC * 2 * SCOL * HH2], BF16, tag="Sst")
            Sstv = Sst[:].rearrange("n (b e p h) -> n b e p h", b=BLOC, e=2,
                                    p=SCOL)
            # scan tiles, lane-packed: row = n + 64*(h//32); re and im
            SA = big.tile([2 * N, BLOC * C2 * HH2], BF16, tag="SA")
            SAv = SA[:].rearrange("n (b g r h) -> n b g r h", b=BLOC, g=NG, r=G1)
            SB = big.tile([2 * N, BLOC * C2 * HH2], BF16, tag="SB")
            SBv = SB[:].rearrange("n (b g r h) -> n b g r h", b=BLOC, g=NG, r=G1)

            for b in range(BLOC):   # zero pad columns (both halves)
                for e in range(2):
                    nc.vector.memset(Sstv[:, b, e, 0, :], 0.0)

            # ---------------- parameters ----------------
            # x-path params on the SP queue (needed first); big D-phase
            # params issued later on the Pool/SWDGE queue (bypasses HWDGE).
            wb_sb = cpar.tile([H + 1, H], BF16, tag="wb")
            nc.sync.dma_start(wb_sb[:], WB.ap())

            # FiLM prologue -> per-(h,b) scalar columns
            fwt = cpar.tile([CD, 2 * H], FP32, tag="fwt")
            nc.gpsimd.dma_start(fwt[:], film_WT.ap())
            fbl = cpar.tile([1, 2 * H], FP32, tag="fbl")
            nc.gpsimd.dma_start(fbl[:], film_bl.ap())
            ct = cpar.tile([CD, BLOC], FP32, tag="ct")
            nc.gpsimd.dma_start(ct[:], condT.ap())
            on1 = cpar.tile([1, BLOC], FP32, tag="on1")
            nc.gpsimd.dma_start(on1[:], ones1.ap())
            gps_t = ps_d.tile([128, 2 * T], FP32, tag="pd")
            gps = gps_t[0:2 * H, 0:BLOC]
            nc.tensor.matmul(gps, fwt[:], ct[:], start=True, stop=False)
            nc.tensor.matmul(gps, fbl[:], on1[:], start=False, stop=True)
            gb_sb = cpar.tile([2 * H, BLOC], FP32, tag="gb")
            nc.scalar.copy(gb_sb[:], gps)
            # film rows are host-permuted to (e, q, hp) order, so this store
            # lands directly in the paired [e, (q, hp, b)] layout
            nc.gpsimd.dma_start(
                gb_scratch.ap().rearrange("e (f b) -> (e f) b", b=BLOC),
                gb_sb[:])
            # paired broadcast columns: col (q, hp, b) rows 0:64 = gb[q,2hp,b],
            # rows 64:128 = gb[q,2hp+1,b]
            gbP = cpar.tile([128, 2 * (H // 2) * BLOC], FP32, tag="gbP")
            nc.gpsimd.dma_start(
                gbP[0:N, :], gb_scratch.ap()[0][None, :]
                .broadcast_to([N, 2 * (H // 2) * BLOC]))
            nc.gpsimd.dma_start(
                gbP[N:128, :], gb_scratch.ap()[1][None, :]
                .broadcast_to([N, 2 * (H // 2) * BLOC]))

            # ---------------- phase A: u = gelu(W x + b) ----------------
            def phase_a(b):
                XC = 8      # chunks per x DMA; 4 chunks per psum tile
                for cg in range(C // XC):
                    xt = xa.tile([H + 1, XC * T], BF16, tag="xt")
                    nc.sync.dma_start(
                        xt[:], xbf.ap()[b, :, cg * XC * T:(cg + 1) * XC * T])
                    for half in range(2):
                        pa = ps_a.tile([T, 4 * H], FP32)
                        for cc in range(4):
                            cx = half * 4 + cc
                            nc.tensor.matmul(
                                pa[:, cc * H:(cc + 1) * H],
                                xt[:, cx * T:(cx + 1) * T], wb_sb[:],
                                start=True, stop=True, skip_group_check=True)
                        c0 = cg * XC + half * 4
                        dst = uv[:, :, b, c0:c0 + 4].rearrange("t h c -> t c h")
                        nc.scalar.activation(dst, pa[:], AF.Gelu)

            # B-phase params (needed right after A(b=0)): Pool queue
            va_sb = {}

            def load_bparams():
                for nm, tens in (("lr", VA_lo_re), ("hr", VA_hi_re),
                                 ("li", VA_lo_im), ("hi", VA_hi_im)):
                    t_ = cpar.tile([T, H * N], BF16, tag="va" + nm)
                    nc.gpsimd.dma_start(
                        t_[:].rearrange("t (h f) -> t h f", h=H), tens.ap())
                    va_sb[nm] = t_

            mu2 = cpar.tile([2 * N, 2 * HH2], BF16, tag="mu2")
            nc.gpsimd.dma_start(mu2[:, 0:HH2], MU2_re.ap())
            nc.gpsimd.dma_start(mu2[:, HH2:], MU2_im.ap())
            nu2 = cpar.tile([2 * N, 2 * HH2], BF16, tag="nu2")
            nc.gpsimd.dma_start(nu2[:, 0:HH2], NU2_re.ap())
            nc.gpsimd.dma_start(nu2[:, HH2:], NU2_im.ap())
            cb = cpar.tile([2 * N, 2 * (G1 - 1) * HH2], BF16, tag="cb")
            cbv = cb[:].rearrange("n (q r h) -> n q r h", q=2, r=G1 - 1)
            nc.gpsimd.dma_start(cbv[:, 0], CB_re.ap())
            nc.gpsimd.dma_start(cbv[:, 1], CB_im.ap())

            # D-phase params: SP queue AFTER all x loads (in-order queue
            # naturally deprioritizes them below phase A's inputs)
            kt_sb = cpar.tile([T, H * T], BF16, tag="kt")
            kt2_sb = cpar.tile([T, H * T], BF16, tag="kt2")
            pst_sb = cpar.tile([2 * N, H * T], BF16, tag="pst")
            pmt_sb = cpar.tile([2 * N, H * T], BF16, tag="pmt")

            def load_dparams():
                nc.sync.dma_start(
                    kt_sb[:].rearrange("t (h f) -> t h f", h=H), KT.ap())
                nc.sync.dma_start(
                    kt2_sb[:].rearrange("t (h f) -> t h f", h=H), KT2.ap())
                nc.sync.dma_start(
                    pst_sb[:].rearrange("n (h f) -> n h f", h=H), PST.ap())
                nc.sync.dma_start(
                    pmt_sb[:].rearrange("n (h f) -> n h f", h=H), PMT.ap())

            # ---------------- per-b: phase B + scan + phase D ----------------
            mre = mu2[:, 0:HH2]; mim = mu2[:, HH2:]
            nre = nu2[:, 0:HH2]; nim = nu2[:, HH2:]

            def cfma(dre, dim_, wre, wim, sre, sim_, fshape):
                """d += w * s (complex); w broadcast tiles, all bf16."""
                nf = int(np.prod(fshape))
                t1 = tmp.tile([2 * N, NG * HH2], BF16, tag="t1")
                t2 = tmp.tile([2 * N, NG * HH2], BF16, tag="t2")
                a = t1[:, :nf].rearrange("n (x y) -> n x y", x=fshape[0])
                bq = t2[:, :nf].rearrange("n (x y) -> n x y", x=fshape[0])
                nc.vector.tensor_mul(a, wre, sre)
                nc.vector.tensor_mul(bq, wim, sim_)
                nc.vector.tensor_sub(a, a, bq)
                nc.vector.tensor_add(dre, dre, a)
                nc.vector.tensor_mul(a, wre, sim_)
                nc.vector.tensor_mul(bq, wim, sre)
                nc.vector.tensor_add(a, a, bq)
                nc.vector.tensor_add(dim_, dim_, a)

            def bphase(b):
                # ---- phase B: V pair-summaries (lane-packed by h-half) ----
                for hg in range(HH2 // HGB):
                    h0 = hg * HGB
                    pv = ps_b.tile([2 * N, 2 * HGB * C2], FP32)   # (q, i, p)
                    for hh in range(HGB):
                        for e in range(2):
                            h = e * HH2 + h0 + hh
                            rows = slice(e * N, e * N + N)
                            rhs_e = uq[:, h, b, :, 0]
                            rhs_o = uq[:, h, b, :, 1]
                            for q, (lo, hi) in enumerate(
                                    (("lr", "hr"), ("li", "hi"))):
                                sl = slice((q * HGB + hh) * C2,
                                           (q * HGB + hh + 1) * C2)
                                vlo = va_sb[lo][:, h * N:(h + 1) * N]
                                vhi = va_sb[hi][:, h * N:(h + 1) * N]
                                nc.tensor.matmul(pv[rows, sl], vlo, rhs_e,
                                                 start=True, stop=False,
                                                 skip_group_check=True)
                                nc.tensor.matmul(pv[rows, sl], vhi, rhs_o,
                                                 start=False, stop=True,
                                                 skip_group_check=True)
                    pvv = pv[:].rearrange("n (q h p) -> n q h p", q=2, h=HGB)
                    dvr = SAv[:, b, :, :, h0:h0 + HGB].rearrange(
                        "n g r h -> n h (g r)")
                    nc.scalar.activation(dvr, pvv[:, 0], AF.Copy)
                    dvi = SBv[:, b, :, :, h0:h0 + HGB].rearrange(
                        "n g r h -> n h (g r)")
                    nc.scalar.activation(dvi, pvv[:, 1], AF.Copy)

            def bscan(b):
                # ---- hierarchical scan over 64 pairs ----
                sreV = SAv[:, b]
                simV = SBv[:, b]
                for r in range(1, G1):
                    cfma(sreV[:, :, r, :], simV[:, :, r, :],
                         mre[:, None, :].broadcast_to([2 * N, NG, HH2]),
                         mim[:, None, :].broadcast_to([2 * N, NG, HH2]),
                         sreV[:, :, r - 1, :], simV[:, :, r - 1, :],
                         (NG, HH2))
                for g in range(1, NG):
                    cfma(sreV[:, g, G1 - 1, :][:, None, :],
                         simV[:, g, G1 - 1, :][:, None, :],
                         nre[:, None, :].broadcast_to([2 * N, 1, HH2]),
                         nim[:, None, :].broadcast_to([2 * N, 1, HH2]),
                         sreV[:, g - 1, G1 - 1, :][:, None, :],
                         simV[:, g - 1, G1 - 1, :][:, None, :],
                         (1, HH2))
                for r in range(G1 - 1):
                    cfma(sreV[:, 1:NG, r, :], simV[:, 1:NG, r, :],
                         cbv[:, 0, r][:, None, :].broadcast_to(
                             [2 * N, NG - 1, HH2]),
                         cbv[:, 1, r][:, None, :].broadcast_to(
                             [2 * N, NG - 1, HH2]),
                         sreV[:, 0:NG - 1, G1 - 1, :],
                         simV[:, 0:NG - 1, G1 - 1, :],
                         (NG - 1, HH2))

            def bstack(b):
                # ---- gather into stacked carry layout (SBUF->SBUF DMAs,
                # issued on the DVE queue so they never block SP's x/rx) ----
                sa = SAv[:, b].rearrange("n g r h -> n (g r h)")
                sb_ = SBv[:, b].rearrange("n g r h -> n (g r h)")
                for e in range(2):
                    nc.gpsimd.dma_start(
                        Sstv[0:N, b, e, 1:SCOL, :].rearrange("n p h -> n (p h)"),
                        sa[e * N:(e + 1) * N, :])
                    nc.gpsimd.dma_start(
                        Sstv[N:2 * N, b, e, 1:SCOL, :].rearrange(
                            "n p h -> n (p h)"),
                        sb_[e * N:(e + 1) * N, :])

            def dphase(b):
                # ---- phase D: conv + carry + FiLM + residual + store ----
                # h-pairs share one [128, 256] psum tile: even h rows 0:64,
                # odd h rows 64:128; one act per pair (gbP columns)
                HP = HG // 2     # 4 pairs per store group
                for hg in range(H // HG):
                    hp0 = hg * HP
                    yt = ytp.tile([128, HP * 2 * T], BF16, tag="yt")
                    rx = rxp.tile([128, HP * 2 * T], BF16, tag="rx")
                    nc.sync.dma_start(
                        rx[:].rearrange("p (h f) -> p h f", h=HP),
                        xrt.ap()[b, :, hp0:hp0 + HP, :])
                    for hh in range(HP):
                        hp = hp0 + hh
                        pdt = ps_d.tile([128, 2 * T], FP32, tag="pd")
                        for e in range(2):
                            h = 2 * hp + e
                            pd = pdt[64 * e:64 * (e + 1), :]
                            lhs_e = uq[:, h, b, :, 0]
                            lhs_o = uq[:, h, b, :, 1]
                            kth = kt_sb[:, h * T:(h + 1) * T]
                            kt2h = kt2_sb[:, h * T:(h + 1) * T]
                            psth = pst_sb[:, h * T:(h + 1) * T]
                            pmth = pmt_sb[:, h * T:(h + 1) * T]
                            ssth = Sstv[:, b, h // HH2, 0:C2, h % HH2]
                            nc.tensor.matmul(pd[:, 0:T], lhs_e, kth,
                                             start=True, stop=False,
                                             skip_group_check=True)
                            nc.tensor.matmul(pd[:, 0:T], ssth, psth,
                                             start=False, stop=True,
                                             skip_group_check=True)
                            nc.tensor.matmul(pd[:, T:2 * T], lhs_o, kth,
                                             start=True, stop=False,
                                             skip_group_check=True)
                            nc.tensor.matmul(pd[:, T:2 * T], lhs_e, kt2h,
                                             start=False, stop=False,
                                             skip_group_check=True)
                            nc.tensor.matmul(pd[:, T:2 * T], ssth, pmth,
                                             start=False, stop=True,
                                             skip_group_check=True)
                        sc = (0 * (H // 2) + hp) * BLOC + b
                        bc = (1 * (H // 2) + hp) * BLOC + b
                        nc.scalar.activation(
                            yt[:, hh * 2 * T:(hh + 1) * 2 * T], pdt[:],
                            AF.Gelu,
                            bias=gbP[:, bc:bc + 1], scale=gbP[:, sc:sc + 1])
                    eng = nc.gpsimd if b == 0 else nc.vector
                    eng.tensor_add(yt[:], yt[:], rx[:])
                    nc.sync.dma_start(
                        y_out.ap()[b].rearrange(
                            "(hp e) (p f) -> e p hp f", e=2, f=2 * T)
                        [:, :, hp0:hp0 + HP, :],
                        yt[:].rearrange("p (h f) -> p h f", h=HP))

            # pipeline: scan(b) on DVE overlaps A/B/D work of the other b;
            # issue order doubles as per-engine queue priority
            phase_a(0)
            load_bparams()
            bphase(0)
            phase_a(1)
            load_dparams()
            bscan(0)
            bstack(0)
            bphase(1)
            bscan(1)
            dphase(0)
            bstack(1)
            dphase(1)

    _split_tail_drain_waits(nc)
    return nc


def kernel(**inputs):
    import ml_dtypes
    if "k" not in _CACHE:
        _CACHE["k"] = _build()
    nc = _CACHE["k"]

    hp = _host_params(
        inputs["log_dt"], inputs["log_A_real"], inputs["A_imag"],
        inputs["C_re"], inputs["C_im"], inputs["D"],
        inputs["W_lin"], inputs["b_lin"])

    x = np.asarray(inputs["x"], dtype=np.float32)
    res_w = np.asarray(inputs["res_w"], dtype=np.float32)
    cond = np.ascontiguousarray(
        np.asarray(inputs["conditional_information"], dtype=np.float32))
    film_W = np.asarray(inputs["film_W"], dtype=np.float32)
    film_b = np.asarray(inputs["film_b"], dtype=np.float32)

    bf16 = ml_dtypes.bfloat16
    xb = x.astype(bf16)                                       # [B,H,L]
    ones_row = np.ones((B, 1, L), dtype=bf16)
    xbf = np.ascontiguousarray(np.concatenate([xb, ones_row], axis=1))
    rx = (x * res_w[None, :, None]).astype(bf16)
    # [b, hp, e, c2, f] -> [b, (e c2), hp, f]
    xrt = np.ascontiguousarray(
        rx.reshape(B, H // 2, 2, C2, 2 * T).transpose(0, 2, 3, 1, 4)
        .reshape(B, 2 * C2, H // 2, 2 * T))

    # permute FiLM output rows to (e, q, hp): new[e*H + q*H/2 + hp] = old[q*H + 2hp + e]
    perm = np.array([q * H + 2 * hp_ + e
                     for e in range(2) for q in range(2)
                     for hp_ in range(H // 2)])
    common = dict(hp)
    common["film_WT"] = np.ascontiguousarray(film_W.T[:, perm])
    common["film_bl"] = np.ascontiguousarray(film_b[None, perm])
    common["ones1"] = np.ones((1, BLOC), np.float32)

    in_maps = []
    for c_ in range(NCORES):
        m = dict(common)
        m["xbf"] = np.ascontiguousarray(xbf[c_ * BLOC:(c_ + 1) * BLOC])
        m["xrt"] = np.ascontiguousarray(xrt[c_ * BLOC:(c_ + 1) * BLOC])
        m["condT"] = np.ascontiguousarray(cond[c_ * BLOC:(c_ + 1) * BLOC].T)
        in_maps.append(m)

    res = run_bass_kernel_spmd(nc, in_maps, core_ids=list(range(NCORES)))
    out = np.concatenate(
        [np.asarray(res.results[c_]["y_out"]) for c_ in range(NCORES)], axis=0)
    return out.astype(np.float32)
